# revision 31
# baseline (speedup 1.0000x reference)
"""GCN (2-layer, PyG GCNConv semantics) on 8 Trainium2 NeuronCores.

V2 strategy (dst-sharded message passing, fp8 front end):
  out = softmax( A @ relu(A @ (x W1) + b1) @ W2 + b2 ),  A = D^-1/2 (Adj+I) D^-1/2

  - Host: degrees/dinv, self-loops appended as ordinary edges, edges
    partitioned by destination core (6250 dst rows per core), each core's
    dst nodes permuted into 50 load-balanced blocks of 128.  Per-edge
    gather indices (int16, SWDGE wrapped) and compact per-slot dst-position
    streams (int16) are precomputed on host (cached to /tmp keyed on the
    edge list).
  - Phase 0 (on-device, redundant per core): z1 = (dinv*x) @ W1 computed
    in fp8 with DoubleRow perf mode, stored fp8 to local HBM.
  - Phase 1: per-edge dma_gather of fp8 z1 rows (4 SWDGE queues);
    one-hot matrices generated ON CHIP (iota + is_equal, DVE) instead of
    streamed; segment-sum via fp8 DoubleRow TensorE matmuls; bias+relu;
    z2 = dinv * (h @ W2) in bf16 per block.
  - AllGather of bf16 z2 across the 8 cores in two row-slices,
    overlapped with phase-1/2 compute.
  - Phase 2: per-edge dma_gather of bf16 z2 rows, on-chip one-hot
    segment-sum, + b2, softmax, DMA out.

kernel(**inputs) -> np.ndarray is self-contained (shapes hardcoded).
"""

import hashlib
import os
import sys
import types

sys.path.insert(0, "/opt/trn_rl_repo")

import numpy as np
import ml_dtypes

from concourse import bass, mybir, bacc, tile
from concourse.bass import broadcast_tensor_aps
from concourse.bass_utils import run_bass_kernel_spmd

BF16 = ml_dtypes.bfloat16
FP8 = ml_dtypes.float8_e4m3

# ---------------- problem constants (hardcoded) ----------------
N_NODES = 50000
D_IN, D_HID, D_OUT = 512, 256, 64
NCORES = 8
RPC = N_NODES // NCORES          # 6250 dst rows per core
BLK = 128
BPC = 50                         # blocks per core (spare slots for balancing)
RPAD = BPC * BLK                 # 6400
NPAD = ((N_NODES + BLK - 1) // BLK) * BLK   # 50048 (391 node blocks)
NBLOCKS = NPAD // BLK            # 391
SPLIT1 = 24960                   # L1 gather src split (block-aligned, int16-safe)
S0_ROWS = 3200                   # AG slice 0: perm positions [0, 3200) = 25 blocks
S1_ROWS = RPAD - S0_ROWS         # 3200: positions [3200, 6400) = 25 blocks
S0_BLOCKS = S0_ROWS // BLK       # 25
PIECE = 256                      # gather slots per dma_gather call (ring-safe)
CHPP = PIECE // BLK              # chunks per piece (2)
NQ = 4                           # SWDGE queues (ucode max)

LAST = {}                        # test harness introspection


def _install_trace_hook():
    try:
        mod = types.ModuleType("antenv.axon_hooks")
        hook = [None]
        mod.set_axon_ntff_profile_hook = lambda h: hook.__setitem__(0, h)
        mod.get_axon_ntff_profile_hook = lambda: hook[0]
        sys.modules["antenv.axon_hooks"] = mod
        import antenv
        antenv.axon_hooks = mod
        from trn_agent_boot.trn_boot import _ntff_profile_via_ctypes
        mod.set_axon_ntff_profile_hook(
            _ntff_profile_via_ctypes("/opt/axon/libaxon_pjrt.so"))
        return True
    except Exception:
        return False


# ---------------- host-side preprocessing ----------------

def _pack_greedy(node_ids, cnts, block_ids, cap):
    """Greedy k-dim balanced packing of node_ids into block_ids (<=128 each).
    cnts: [ndim, RPC] per-node counts. Returns {node: block}."""
    nd = len(cnts)
    nb = len(block_ids)
    tot = sum(c[node_ids] for c in cnts)
    order = node_ids[np.argsort(-tot, kind="stable")]
    sums = np.zeros((nd, nb), dtype=np.float64)
    cnt = np.zeros(nb, dtype=np.int64)
    assign = {}
    big = 1e18
    for i in order:
        score = np.max([(sums[d] + cnts[d][i]) / cap for d in range(nd)], axis=0)
        score = score + (sums.sum(axis=0) + tot[0] * 0) * 1e-7
        score = np.where(cnt < BLK, score, big)
        j = int(np.argmin(score))
        assign[i] = j
        cnt[j] += 1
        for d in range(nd):
            sums[d, j] += cnts[d][i]
    # repair per dim
    members = {j: [i for i, jj in assign.items() if jj == j] for j in range(nb)}
    for d in range(nd):
        for _ in range(2000):
            j = int(np.argmax(sums[d]))
            if sums[d, j] <= cap:
                break
            ms = members[j]
            pos_m = [i for i in ms if cnts[d][i] > 0]
            if not pos_m:
                break
            mv = min(pos_m, key=lambda i: cnts[d][i])
            tgt = np.where(cnt < BLK, sums[d], big)
            tgt[j] = big
            jt = int(np.argmin(tgt))
            if tgt[jt] >= big:
                break
            assign[mv] = jt
            members[j].remove(mv)
            members[jt].append(mv)
            cnt[j] -= 1
            cnt[jt] += 1
            for dd in range(nd):
                sums[dd, j] -= cnts[dd][mv]
                sums[dd, jt] += cnts[dd][mv]
    return assign


def _positions_from_assign(assign, block_ids):
    pos = {}
    slot = {j: 0 for j in block_ids}
    for i in sorted(assign):
        j = assign[i]
        pos[i] = j * BLK + slot[j]
        slot[j] += 1
    return pos


def _pack_blocks(cntA, cntB, cap=1148):
    nodes = np.arange(RPC)
    assign = _pack_greedy(nodes, [cntA, cntB], list(range(BPC)), cap)
    posd = _positions_from_assign(assign, list(range(BPC)))
    pos = np.empty(RPC, dtype=np.int64)
    for i in range(RPC):
        pos[i] = posd[i]
    return pos


def _pack_blocks4(cntA, cntB, cntC, cntD, half0_nodes, cap=1148):
    """Second pass: rebalance within halves on 4 dims."""
    pos = np.empty(RPC, dtype=np.int64)
    all_nodes = np.arange(RPC)
    h0 = half0_nodes
    h1 = all_nodes[~np.isin(all_nodes, h0)]
    for nodes, blocks in ((h0, list(range(S0_BLOCKS))),
                          (h1, list(range(S0_BLOCKS, BPC)))):
        assign = _pack_greedy(nodes, [cntA, cntB, cntC, cntD], blocks, cap)
        posd = {}
        slot = {j: 0 for j in range(len(blocks))}
        for i in sorted(assign):
            j = assign[i]
            posd[i] = blocks[j] * BLK + slot[j]
            slot[j] += 1
        for i in nodes:
            pos[i] = posd[i]
    return pos


def _build_stream(e_pos, e_idx16, K, e_par=None):
    """Returns (idx_wrapped [128, SL/16] i16, pos... ).
    pos[p, ci] = dst position within block for slot ci*128+p, 255 for pads.
    With e_par (0/1 per edge): returns parity-split (pos_even, pos_odd)."""
    nch = BPC * K
    SL = nch * BLK
    blk = e_pos // BLK
    o = np.argsort(blk, kind="stable")
    blk_s = blk[o]
    e_pos = e_pos[o]
    e_idx16 = e_idx16[o]
    if e_par is not None:
        e_par = e_par[o]
    counts = np.bincount(blk_s, minlength=BPC)
    assert counts.max() <= K * BLK, (counts.max(), K * BLK)
    starts = np.concatenate([[0], np.cumsum(counts)[:-1]])
    within = np.arange(len(blk_s)) - np.repeat(starts, counts)
    slot = blk_s * (K * BLK) + within

    idx_full = np.zeros(SL, dtype=np.int16)
    idx_full[slot] = e_idx16
    idx_w = np.tile(idx_full.reshape(SL // 16, 16).T, (8, 1)).copy()

    if e_par is None:
        pos_full = np.full(SL, 255, dtype=np.int16)
        pos_full[slot] = (e_pos % BLK).astype(np.int16)
        pos = pos_full.reshape(nch, BLK).T.copy()
        return idx_w, pos
    pe = np.full(SL, 255, dtype=np.int16)
    po = np.full(SL, 255, dtype=np.int16)
    pv = (e_pos % BLK).astype(np.int16)
    pe[slot[e_par == 0]] = pv[e_par == 0]
    po[slot[e_par == 1]] = pv[e_par == 1]
    return (idx_w, pe.reshape(nch, BLK).T.copy(),
            po.reshape(nch, BLK).T.copy())


def _preprocess_structure(edge_index):
    """Graph-structure-only preprocessing (cacheable)."""
    src = np.asarray(edge_index[0], dtype=np.int64)
    dst = np.asarray(edge_index[1], dtype=np.int64)
    loops = np.arange(N_NODES, dtype=np.int64)
    src_all = np.concatenate([src, loops])
    dst_all = np.concatenate([dst, loops])
    deg = np.bincount(dst_all, minlength=N_NODES).astype(np.float32)
    dinv = np.where(deg > 0, 1.0 / np.sqrt(deg), 0.0).astype(np.float32)

    core_of = dst_all // RPC

    perms = []
    core_edges = []
    cnts_ab = []
    for c in range(NCORES):
        m = core_of == c
        s_c = src_all[m]
        d_loc = (dst_all[m] - c * RPC).astype(np.int64)
        cntA = np.bincount(d_loc[s_c < SPLIT1], minlength=RPC)
        cntB = np.bincount(d_loc[s_c >= SPLIT1], minlength=RPC)
        perms.append(_pack_blocks(cntA, cntB))
        core_edges.append((s_c, d_loc))
        cnts_ab.append((cntA, cntB))

    permpos_global = np.empty(N_NODES, dtype=np.int64)
    for c in range(NCORES):
        permpos_global[c * RPC:(c + 1) * RPC] = perms[c]

    # pass 2: rebalance within halves, also evening C/D (src-half) counts
    half_global = permpos_global < S0_ROWS
    perms2 = []
    for c in range(NCORES):
        s_c, d_loc = core_edges[c]
        cntA, cntB = cnts_ab[c]
        hsrc = half_global[s_c]
        cntC = np.bincount(d_loc[hsrc], minlength=RPC)
        cntD = np.bincount(d_loc[~hsrc], minlength=RPC)
        half0_nodes = np.where(perms[c] < S0_ROWS)[0]
        perms2.append(_pack_blocks4(cntA, cntB, cntC, cntD, half0_nodes))
    perms = perms2
    for c in range(NCORES):
        permpos_global[c * RPC:(c + 1) * RPC] = perms[c]

    def seg_K(e_pos):
        counts = np.bincount(e_pos // BLK, minlength=BPC)
        return int(np.ceil(counts.max() / BLK))

    K1A = K1B = K2C = K2D = 1
    meta = []
    for c in range(NCORES):
        s_c, d_loc = core_edges[c]
        pos_d = perms[c][d_loc]
        mA = s_c < SPLIT1
        src_r = s_c // RPC
        src_pos = permpos_global[s_c]   # core-local position (0..RPAD-1)
        mC = src_pos < S0_ROWS
        K1A = max(K1A, seg_K(pos_d[mA]))
        K1B = max(K1B, seg_K(pos_d[~mA]))
        K2C = max(K2C, seg_K(pos_d[mC]))
        K2D = max(K2D, seg_K(pos_d[~mC]))
        meta.append((s_c, d_loc, pos_d, mA, mC, src_r, src_pos))

    streams = []
    for c in range(NCORES):
        s_c, d_loc, pos_d, mA, mC, src_r, src_pos = meta[c]
        i1a, p1a = _build_stream(pos_d[mA], s_c[mA].astype(np.int16), K1A)
        i1b, p1b = _build_stream(pos_d[~mA],
                                 (s_c[~mA] - SPLIT1).astype(np.int16), K1B)
        # phase-2 rows gathered in PAIRS (256B descriptors): idx = row//2,
        # the row parity selects the 64-col half of the gathered 128 cols.
        idxC = (src_r * S0_ROWS + src_pos).astype(np.int64)
        idxD = (src_r * S1_ROWS + (src_pos - S0_ROWS)).astype(np.int64)
        i2c, p2ce, p2co = _build_stream(
            pos_d[mC], (idxC[mC] // 2).astype(np.int16), K2C,
            e_par=(idxC[mC] % 2))
        i2d, p2de, p2do = _build_stream(
            pos_d[~mC], (idxD[~mC] // 2).astype(np.int16), K2D,
            e_par=(idxD[~mC] % 2))

        dinvb = np.zeros((BLK, BPC), dtype=np.float32)
        nodes_at = np.full(RPAD, -1, dtype=np.int64)
        nodes_at[perms[c]] = np.arange(RPC)
        valid = nodes_at >= 0
        dv = np.zeros(RPAD, np.float32)
        dv[valid] = dinv[nodes_at[valid] + c * RPC]
        dinvb[:, :] = dv.reshape(BPC, BLK).T

        streams.append({
            "i1a": i1a, "p1a": p1a, "i1b": i1b, "p1b": p1b,
            "i2c": i2c, "p2ce": p2ce, "p2co": p2co,
            "i2d": i2d, "p2de": p2de, "p2do": p2do,
            "dinvb": dinvb,
        })
    permarr = np.stack(perms)
    return {
        "streams": streams, "perms": permarr, "dinv": dinv,
        "Ks": np.array([K1A, K1B, K2C, K2D]),
    }


def _cached_structure(edge_index):
    ei = np.ascontiguousarray(np.asarray(edge_index, dtype=np.int64))
    h = hashlib.md5(ei.tobytes()).hexdigest()[:16]
    path = f"/tmp/gcn_pre_{h}_v4.npz"
    try:
        if os.path.exists(path):
            z = np.load(path)
            streams = []
            for c in range(NCORES):
                streams.append({k: z[f"{k}_{c}"] for k in
                                ("i1a", "p1a", "i1b", "p1b",
                                 "i2c", "p2ce", "p2co",
                                 "i2d", "p2de", "p2do", "dinvb")})
            return {"streams": streams, "perms": z["perms"],
                    "dinv": z["dinv"], "Ks": z["Ks"]}
    except Exception:
        pass
    st = _preprocess_structure(edge_index)
    try:
        flat = {"perms": st["perms"], "dinv": st["dinv"], "Ks": st["Ks"]}
        for c in range(NCORES):
            for k, v in st["streams"][c].items():
                flat[f"{k}_{c}"] = v
        np.savez(path + ".tmp.npz", **flat)
        os.replace(path + ".tmp.npz", path)
    except Exception:
        pass
    return st


def _preprocess(x, edge_index, W1, b1, W2, b2):
    st = _cached_structure(edge_index)
    dinv = st["dinv"]
    K1A, K1B, K2C, K2D = (int(v) for v in st["Ks"])

    xs = (np.asarray(x, np.float32) * dinv[:, None])
    GRP = 16
    NGRP = (NBLOCKS + GRP - 1) // GRP      # 25
    NPAD2 = NGRP * GRP * BLK               # 51200
    xT = np.zeros((D_IN, NPAD2), dtype=FP8)
    xT[:, :N_NODES] = xs.T.astype(FP8)
    # pre-tiled contiguous layout: [group, partition, kchunk, cols]
    xtg = np.ascontiguousarray(
        xT.reshape(4, 128, NGRP, GRP * BLK).transpose(2, 1, 0, 3))
    w1q = np.asarray(W1, np.float32).astype(FP8)
    b1rep = np.tile(np.asarray(b1, np.float32)[None, :], (128, 1)).copy()
    ident = np.eye(128, dtype=np.float32).astype(BF16)
    b2rep = np.tile(np.asarray(b2, np.float32)[None, :], (128, 1)).copy()
    w2b = np.asarray(W2, np.float32).astype(BF16)

    in_maps = []
    for c in range(NCORES):
        s = st["streams"][c]
        in_maps.append({
            "xTg": xtg, "w1": w1q, "w2": w2b, "b1rep": b1rep, "ident": ident,
            "b2rep": b2rep, "dinvb": np.ascontiguousarray(s["dinvb"]),
            "i1a": s["i1a"], "p1a": s["p1a"],
            "i1b": s["i1b"], "p1b": s["p1b"],
            "i2c": s["i2c"], "p2ce": s["p2ce"], "p2co": s["p2co"],
            "i2d": s["i2d"], "p2de": s["p2de"], "p2do": s["p2do"],
        })

    real = 800000 + N_NODES
    padded = NCORES * BLK * BPC * (K1A + K1B)
    LAST["K"] = (K1A, K1B, K2C, K2D)
    LAST["pad_frac"] = padded / real - 1.0
    perms = [st["perms"][c] for c in range(NCORES)]
    return in_maps, perms, (K1A, K1B, K2C, K2D)


# ---------------- device program ----------------

def _build_program(K1A, K1B, K2C, K2D):
    dt = mybir.dt
    DR = mybir.MatmulPerfMode.DoubleRow
    phases = int(os.environ.get("GCN_PHASES", "3"))
    nc = bacc.Bacc(None, target_bir_lowering=False, debug=False,
                   num_devices=NCORES, num_swdge_queues=NQ)

    GRP = 16
    NGRP = (NBLOCKS + GRP - 1) // GRP
    xTg = nc.dram_tensor("xTg", [NGRP, 128, 4, GRP * BLK], dt.float8e4,
                         kind="ExternalInput")
    w1 = nc.dram_tensor("w1", [D_IN, D_HID], dt.float8e4, kind="ExternalInput")
    w2 = nc.dram_tensor("w2", [D_HID, D_OUT], dt.bfloat16, kind="ExternalInput")
    b1rep = nc.dram_tensor("b1rep", [128, D_HID], dt.float32, kind="ExternalInput")
    ident = nc.dram_tensor("ident", [128, 128], dt.bfloat16, kind="ExternalInput")
    b2rep = nc.dram_tensor("b2rep", [128, D_OUT], dt.float32, kind="ExternalInput")
    dinvb = nc.dram_tensor("dinvb", [128, BPC], dt.float32, kind="ExternalInput")

    def idx_t(name, K):
        return nc.dram_tensor(name, [128, BPC * K * BLK // 16], dt.int16,
                              kind="ExternalInput")

    def pos_t(name, K):
        return nc.dram_tensor(name, [128, BPC * K], dt.int16,
                              kind="ExternalInput")

    i1a, p1a = idx_t("i1a", K1A), pos_t("p1a", K1A)
    i1b, p1b = idx_t("i1b", K1B), pos_t("p1b", K1B)
    i2c = idx_t("i2c", K2C)
    p2ce, p2co = pos_t("p2ce", K2C), pos_t("p2co", K2C)
    i2d = idx_t("i2d", K2D)
    p2de, p2do = pos_t("p2de", K2D), pos_t("p2do", K2D)

    out = nc.dram_tensor("out", [RPAD, D_OUT], dt.float32, kind="ExternalOutput")

    z1A = nc.dram_tensor("z1A", [SPLIT1, D_HID], dt.float8e4)
    z1B = nc.dram_tensor("z1B", [NPAD - SPLIT1, D_HID], dt.float8e4)
    # z2 rows unpadded (64 cols bf16 = 128B); the gather fetches PAIRS of
    # rows per 256B descriptor via a [rows/2, 128] view of these tensors.
    z2in0 = nc.dram_tensor("z2in0", [S0_ROWS, D_OUT], dt.bfloat16)
    z2in1 = nc.dram_tensor("z2in1", [S1_ROWS, D_OUT], dt.bfloat16)
    z2out0 = nc.dram_tensor("z2out0", [NCORES * S0_ROWS, D_OUT], dt.bfloat16,
                            addr_space="Shared")
    z2out1 = nc.dram_tensor("z2out1", [NCORES * S1_ROWS, D_OUT], dt.bfloat16,
                            addr_space="Shared")

    qctr = [0]

    def next_q():
        q = qctr[0] % NQ
        qctr[0] += 1
        return q

    with tile.TileContext(nc) as tc:
        with tc.tile_pool(name="consts", bufs=1) as cp, \
             tc.tile_pool(name="ph0x", bufs=2) as xp, \
             tc.tile_pool(name="ph0o", bufs=3) as op0, \
             tc.tile_pool(name="gp", bufs=8) as gp, \
             tc.tile_pool(name="gp2", bufs=8) as gp2, \
             tc.tile_pool(name="ohp", bufs=3) as ohp, \
             tc.tile_pool(name="csp", bufs=BPC + 1) as csp, \
             tc.tile_pool(name="hp", bufs=2) as hp, \
             tc.tile_pool(name="zp", bufs=3) as zp, \
             tc.tile_pool(name="smp", bufs=8) as smp, \
             tc.tile_pool(name="psAcc", bufs=3, space="PSUM") as psAcc, \
             tc.tile_pool(name="psMisc", bufs=1, space="PSUM") as psMisc, \
             tc.tile_pool(name="psO", bufs=3, space="PSUM") as psO:
            w1t = cp.tile([128, 4, D_HID], dt.float8e4)
            nc.sync.dma_start(
                w1t[:], w1.ap().rearrange("(k p) n -> p k n", p=128))
            w2t = cp.tile([128, 2, D_OUT], dt.bfloat16)
            nc.sync.dma_start(
                w2t[:], w2.ap().rearrange("(k p) n -> p k n", p=128))
            b1r = cp.tile([128, D_HID], dt.float32)
            nc.sync.dma_start(b1r[:], b1rep[:, :])
            idt = cp.tile([128, 128], dt.bfloat16)
            nc.sync.dma_start(idt[:], ident[:, :])
            b2t = cp.tile([128, D_OUT], dt.float32)
            nc.sync.dma_start(b2t[:], b2rep[:, :])
            dvt = cp.tile([128, BPC], dt.float32)
            nc.sync.dma_start(dvt[:], dinvb[:, :])
            it1a = cp.tile([128, BPC * K1A * BLK // 16], dt.int16)
            nc.scalar.dma_start(it1a[:], i1a[:, :])
            it1b = cp.tile([128, BPC * K1B * BLK // 16], dt.int16)
            nc.scalar.dma_start(it1b[:], i1b[:, :])
            it2c = cp.tile([128, BPC * K2C * BLK // 16], dt.int16)
            nc.scalar.dma_start(it2c[:], i2c[:, :])
            it2d = cp.tile([128, BPC * K2D * BLK // 16], dt.int16)
            nc.scalar.dma_start(it2d[:], i2d[:, :])
            pt1a = cp.tile([128, BPC * K1A], dt.int16)
            nc.scalar.dma_start(pt1a[:], p1a[:, :])
            pt1b = cp.tile([128, BPC * K1B], dt.int16)
            nc.scalar.dma_start(pt1b[:], p1b[:, :])
            pt2ce = cp.tile([128, BPC * K2C], dt.int16)
            nc.scalar.dma_start(pt2ce[:], p2ce[:, :])
            pt2co = cp.tile([128, BPC * K2C], dt.int16)
            nc.scalar.dma_start(pt2co[:], p2co[:, :])
            pt2de = cp.tile([128, BPC * K2D], dt.int16)
            nc.scalar.dma_start(pt2de[:], p2de[:, :])
            pt2do = cp.tile([128, BPC * K2D], dt.int16)
            nc.scalar.dma_start(pt2do[:], p2do[:, :])
            iot = cp.tile([128, 128], dt.int16)
            nc.gpsimd.iota(iot[:], [[1, 128]], base=0, channel_multiplier=0)
            iot3 = iot[:].rearrange("p (o j) -> p o j", o=1)

            # ------- phase 0: z1 = xT^T @ W1, fp8 DoubleRow -------
            z1Av = z1A.ap().rearrange("(n p) f -> p n f", p=128)
            z1Bv = z1B.ap().rearrange("(n p) f -> p n f", p=128)
            NB_A = SPLIT1 // BLK
            GB = 7
            for g0 in range(0, NBLOCKS, GRP):
                gb = min(GRP, NBLOCKS - g0)
                xg = xp.tile([128, 4, GRP * BLK], dt.float8e4, tag="xg")
                nc.sync.dma_start(
                    xg[:, :, :gb * BLK],
                    xTg.ap()[g0 // GRP, :, :, :gb * BLK])
                for b0 in range(0, gb, GB):
                    nb = min(GB, gb - b0)
                    zo = op0.tile([128, GB, D_HID], dt.float8e4, tag="zo")
                    for i in range(nb):
                        ps = psAcc.tile([128, D_HID], dt.float32, tag="acc")
                        col = (b0 + i) * BLK
                        for t in range(2):
                            nc.tensor.matmul(
                                ps[:],
                                xg[:, 2 * t:2 * t + 2, col:col + BLK],
                                w1t[:, 2 * t:2 * t + 2, :],
                                start=(t == 0), stop=(t == 1),
                                perf_mode=DR)
                        nc.vector.tensor_copy(zo[:, i, :], ps[:])
                    lo, hi = g0 + b0, g0 + b0 + nb
                    if hi <= NB_A:
                        nc.sync.dma_start(z1Av[:, lo:hi, :], zo[:, :nb, :])
                    elif lo >= NB_A:
                        nc.sync.dma_start(
                            z1Bv[:, lo - NB_A:hi - NB_A, :], zo[:, :nb, :])
                    else:
                        na = NB_A - lo
                        nc.sync.dma_start(z1Av[:, lo:NB_A, :], zo[:, :na, :])
                        nc.sync.dma_start(
                            z1Bv[:, 0:hi - NB_A, :], zo[:, na:nb, :])

            # ---------------- phases 1+2 ----------------
            seg1 = {
                "A": (K1A, it1a, pt1a, z1A.ap()[:, :]),
                "B": (K1B, it1b, pt1b, z1B.ap()[:, :]),
            }
            seg2 = {
                "C": (K2C, it2c, (pt2ce, pt2co),
                      z2out0.ap().rearrange("(r two) f -> r (two f)", two=2)),
                "D": (K2D, it2d, (pt2de, pt2do),
                      z2out1.ap().rearrange("(r two) f -> r (two f)", two=2)),
            }
            gtiles = {}

            def ensure_g(layer, s, pi):
                key = (layer, s, pi)
                if key in gtiles:
                    return gtiles[key]
                K, itile, _, zview = (seg1 if layer == 1 else seg2)[s]
                felem = D_HID if layer == 1 else 2 * D_OUT
                fdt = dt.float8e4 if layer == 1 else dt.bfloat16
                SL = BPC * K * BLK
                n = min(PIECE, SL - pi * PIECE)
                off = pi * (PIECE // 16)
                pool = gp if layer == 1 else gp2
                gt = pool.tile([128, CHPP, felem], fdt, tag=f"g{layer}{s}")
                nc.gpsimd.dma_gather(
                    gt[:, :n // 128, :], zview, itile[:, off:off + n // 16],
                    n, n, felem, queue_num=next_q())
                gtiles[key] = gt
                return gt

            def make_onehot_t(ptile, K, b, odt, tag):
                oh = ohp.tile([128, K, 128], odt, tag=tag)
                pv = ptile[:, b * K:(b + 1) * K].rearrange(
                    "p (c o) -> p c o", o=1)
                a0, a1 = broadcast_tensor_aps(iot3, pv)
                nc.vector.tensor_tensor(
                    oh[:], a0, a1, op=mybir.AluOpType.is_equal)
                return oh

            def make_onehot(layer, s, b):
                K, _, ptile, _ = seg1[s]
                return make_onehot_t(ptile, K, b, dt.float8e4, f"oh{layer}{s}")

            def l1_block(b):
                hps = psAcc.tile([128, D_HID], dt.float32, tag="acc")
                first = True
                for s in ("A", "B"):
                    K = seg1[s][0]
                    oh = make_onehot(1, s, b)
                    k = 0
                    while k < K:
                        ci = b * K + k
                        pi, cpos = divmod(ci, CHPP)
                        last_s = (s == "B")
                        gt = ensure_g(1, s, pi)
                        if k + 1 < K and cpos + 1 < CHPP:
                            nc.tensor.matmul(
                                hps[:], oh[:, k:k + 2, :],
                                gt[:, cpos:cpos + 2, :],
                                start=first, stop=(last_s and k + 2 == K),
                                perf_mode=DR)
                            k += 2
                        else:
                            nc.tensor.matmul(
                                hps[:], oh[:, k, :], gt[:, cpos, :],
                                start=first, stop=(last_s and k + 1 == K))
                            k += 1
                        first = False
                hs = hp.tile([128, D_HID], dt.float32, tag="hs")
                nc.vector.tensor_scalar(
                    hs[:], hps[:], dvt[:, b:b + 1], None,
                    op0=mybir.AluOpType.mult)
                hb = hp.tile([128, D_HID], dt.bfloat16, tag="hb")
                nc.vector.tensor_tensor(
                    hb[:], hs[:], b1r[:], op=mybir.AluOpType.add)
                hr = hp.tile([128, D_HID], dt.bfloat16, tag="hr")
                nc.scalar.activation(
                    hr[:], hb[:], mybir.ActivationFunctionType.Relu)
                hT = hp.tile([128, 2, 128], dt.bfloat16, tag="hT")
                for h in range(2):
                    tps = psMisc.tile([128, 128], dt.bfloat16, tag="tps")
                    nc.tensor.transpose(
                        tps[:], hr[:, h * 128:(h + 1) * 128], idt[:])
                    nc.scalar.copy(hT[:, h, :], tps[:])
                zps = psMisc.tile([128, D_OUT], dt.float32, tag="zps")
                for h in range(2):
                    nc.tensor.matmul(
                        zps[:], hT[:, h, :], w2t[:, h, :],
                        start=(h == 0), stop=(h == 1))
                z2s = zp.tile([128, D_OUT], dt.bfloat16, tag="z2s")
                nc.vector.tensor_scalar(
                    z2s[:], zps[:], dvt[:, b:b + 1], None,
                    op0=mybir.AluOpType.mult)
                if b < S0_BLOCKS:
                    nc.sync.dma_start(
                        z2in0.ap()[b * BLK:(b + 1) * BLK, :], z2s[:])
                else:
                    bb = b - S0_BLOCKS
                    nc.sync.dma_start(
                        z2in1.ap()[bb * BLK:(bb + 1) * BLK, :], z2s[:])

            cstash = {}

            def l2cd_block(b, s):
                ops = psO.tile([128, D_OUT], dt.float32, tag="ops")
                K, _, (pte, pto), _ = seg2[s]
                ohe = make_onehot_t(pte, K, b, dt.bfloat16, f"oh2{s}e")
                oho = make_onehot_t(pto, K, b, dt.bfloat16, f"oh2{s}o")
                for k in range(K):
                    ci = b * K + k
                    pi, cpos = divmod(ci, CHPP)
                    gt = ensure_g(2, s, pi)
                    nc.tensor.matmul(
                        ops[:], ohe[:, k, :], gt[:, cpos, :D_OUT],
                        start=(k == 0), stop=False)
                    nc.tensor.matmul(
                        ops[:], oho[:, k, :], gt[:, cpos, D_OUT:],
                        start=False, stop=(k == K - 1))
                return ops

            def l2c_block(b):
                ops = l2cd_block(b, "C")
                cs0 = zp.tile([128, D_OUT], dt.float32, tag="cs0")
                nc.scalar.activation(
                    cs0[:], ops[:], mybir.ActivationFunctionType.Copy,
                    scale=dvt[:, b:b + 1])
                cs = csp.tile([128, D_OUT], dt.float32, tag="cs")
                nc.vector.tensor_tensor(
                    cs[:], cs0[:], b2t[:], op=mybir.AluOpType.add)
                cstash[b] = cs

            def l2d_block(b):
                ops = l2cd_block(b, "D")
                t = smp.tile([128, D_OUT], dt.float32, tag="t")
                nc.vector.tensor_scalar(
                    t[:], ops[:], dvt[:, b:b + 1], None,
                    op0=mybir.AluOpType.mult)
                t2 = smp.tile([128, D_OUT], dt.float32, tag="t2")
                nc.vector.tensor_tensor(
                    t2[:], t[:], cstash[b][:], op=mybir.AluOpType.add)
                nm = smp.tile([128, 1], dt.float32, tag="nm")
                nc.vector.reduce_max(
                    nm[:], t2[:], axis=mybir.AxisListType.X, negate=True)
                ex = smp.tile([128, D_OUT], dt.float32, tag="ex")
                sm = smp.tile([128, 1], dt.float32, tag="sm")
                nc.scalar.activation(
                    ex[:], t2[:], mybir.ActivationFunctionType.Exp,
                    bias=nm[:], accum_out=sm[:])
                rc = smp.tile([128, 1], dt.float32, tag="rc")
                nc.vector.reciprocal(rc[:], sm[:])
                ot = smp.tile([128, D_OUT], dt.float32, tag="ot")
                nc.vector.tensor_scalar(
                    ot[:], ex[:], rc[:], None, op0=mybir.AluOpType.mult)
                nc.sync.dma_start(out.ap()[b * BLK:(b + 1) * BLK, :], ot[:])

            if phases >= 1:
                for b in range(32):
                    l1_block(b)
                if phases >= 2:
                    nc.gpsimd.collective_compute(
                        "AllGather", mybir.AluOpType.bypass,
                        replica_groups=[list(range(NCORES))],
                        ins=[z2in0.ap().opt()], outs=[z2out0.ap().opt()])
                ci = 0
                for b in range(32, BPC):
                    l1_block(b)
                    if phases >= 3 and b >= 40 and ci < BPC:
                        l2c_block(ci)
                        ci += 1
                if phases >= 2:
                    nc.gpsimd.collective_compute(
                        "AllGather", mybir.AluOpType.bypass,
                        replica_groups=[list(range(NCORES))],
                        ins=[z2in1.ap().opt()], outs=[z2out1.ap().opt()])
                if phases >= 3:
                    while ci < BPC:
                        l2c_block(ci)
                        ci += 1
                    for b in range(BPC):
                        l2d_block(b)

    nc.compile()

    # The Tile scheduler reorders instructions; DMASW sem lanes rotate mod 8
    # over Pool-engine DMAs in FINAL order and each sem may only be driven by
    # one SWDGE queue.  Rewrite queue_num to lane%NQ in final order so the
    # lane<->queue pairing is always consistent.
    if os.environ.get("GCN_QFIX", "1") == "1":
        from concourse.tile_scheduler import DMAInst
        idx = 0
        for blk in nc.m.functions[0].blocks:
            for inst in blk.instructions:
                if (inst.engine == mybir.EngineType.Pool
                        and isinstance(inst, DMAInst)):
                    inst.queue_num = (idx % 8) % NQ
                    idx += 1
    return nc


# ---------------- entry point ----------------

def kernel(x, edge_index, W1, b1, W2, b2):
    x = np.asarray(x)
    edge_index = np.asarray(edge_index)
    in_maps, perms, Ks = _preprocess(x, edge_index, W1, b1, W2, b2)
    nc = _build_program(*Ks)

    trace = os.environ.get("GCN_TRACE", "0") == "1"
    if trace:
        trace = _install_trace_hook()
    res = run_bass_kernel_spmd(
        nc, in_maps, core_ids=list(range(NCORES)), trace=trace)
    LAST["exec_time_ns"] = res.exec_time_ns
    LAST["results"] = res

    out = np.empty((N_NODES, D_OUT), dtype=np.float32)
    for c in range(NCORES):
        oc = np.asarray(res.results[c]["out"], dtype=np.float32)
        out[c * RPC:(c + 1) * RPC] = oc[perms[c]]
    return out


# revision 32
# speedup vs baseline: 1.1300x; 1.1300x over previous
"""GCN (2-layer, PyG GCNConv semantics) on 8 Trainium2 NeuronCores.

V2 strategy (dst-sharded message passing, fp8 front end):
  out = softmax( A @ relu(A @ (x W1) + b1) @ W2 + b2 ),  A = D^-1/2 (Adj+I) D^-1/2

  - Host: degrees/dinv, self-loops appended as ordinary edges, edges
    partitioned by destination core (6250 dst rows per core), each core's
    dst nodes permuted into 50 load-balanced blocks of 128.  Per-edge
    gather indices (int16, SWDGE wrapped) and compact per-slot dst-position
    streams (int16) are precomputed on host (cached to /tmp keyed on the
    edge list).
  - Phase 0 (on-device, redundant per core): z1 = (dinv*x) @ W1 computed
    in fp8 with DoubleRow perf mode, stored fp8 to local HBM.
  - Phase 1: per-edge dma_gather of fp8 z1 rows (4 SWDGE queues);
    one-hot matrices generated ON CHIP (iota + is_equal, DVE) instead of
    streamed; segment-sum via fp8 DoubleRow TensorE matmuls; bias+relu;
    z2 = dinv * (h @ W2) in bf16 per block.
  - AllGather of bf16 z2 across the 8 cores in two row-slices,
    overlapped with phase-1/2 compute.
  - Phase 2: per-edge dma_gather of bf16 z2 rows, on-chip one-hot
    segment-sum, + b2, softmax, DMA out.

kernel(**inputs) -> np.ndarray is self-contained (shapes hardcoded).
"""

import hashlib
import os
import sys
import types

sys.path.insert(0, "/opt/trn_rl_repo")

import numpy as np
import ml_dtypes

from concourse import bass, mybir, bacc, tile
from concourse.bass import broadcast_tensor_aps
from concourse.bass_utils import run_bass_kernel_spmd

BF16 = ml_dtypes.bfloat16
FP8 = ml_dtypes.float8_e4m3

# ---------------- problem constants (hardcoded) ----------------
N_NODES = 50000
D_IN, D_HID, D_OUT = 512, 256, 64
NCORES = 8
RPC = N_NODES // NCORES          # 6250 dst rows per core
BLK = 128
BPC = 50                         # blocks per core (spare slots for balancing)
RPAD = BPC * BLK                 # 6400
NPAD = ((N_NODES + BLK - 1) // BLK) * BLK   # 50048 (391 node blocks)
NBLOCKS = NPAD // BLK            # 391
SPLIT1 = 24960                   # L1 gather src split (block-aligned, int16-safe)
S0_ROWS = 3200                   # AG slice 0: perm positions [0, 3200) = 25 blocks
S1_ROWS = RPAD - S0_ROWS         # 3200: positions [3200, 6400) = 25 blocks
S0_BLOCKS = S0_ROWS // BLK       # 25
PIECE = 512                      # gather slots per dma_gather call (ring-safe)
CHPP = PIECE // BLK              # chunks per piece (4)
NQ = 4                           # SWDGE queues (ucode max)

LAST = {}                        # test harness introspection


def _install_trace_hook():
    try:
        mod = types.ModuleType("antenv.axon_hooks")
        hook = [None]
        mod.set_axon_ntff_profile_hook = lambda h: hook.__setitem__(0, h)
        mod.get_axon_ntff_profile_hook = lambda: hook[0]
        sys.modules["antenv.axon_hooks"] = mod
        import antenv
        antenv.axon_hooks = mod
        from trn_agent_boot.trn_boot import _ntff_profile_via_ctypes
        mod.set_axon_ntff_profile_hook(
            _ntff_profile_via_ctypes("/opt/axon/libaxon_pjrt.so"))
        return True
    except Exception:
        return False


# ---------------- host-side preprocessing ----------------

def _pack_greedy(node_ids, cnts, block_ids, cap):
    """Greedy k-dim balanced packing of node_ids into block_ids (<=128 each).
    cnts: [ndim, RPC] per-node counts. Returns {node: block}."""
    nd = len(cnts)
    nb = len(block_ids)
    tot = sum(c[node_ids] for c in cnts)
    order = node_ids[np.argsort(-tot, kind="stable")]
    sums = np.zeros((nd, nb), dtype=np.float64)
    cnt = np.zeros(nb, dtype=np.int64)
    assign = {}
    big = 1e18
    for i in order:
        score = np.max([(sums[d] + cnts[d][i]) / cap for d in range(nd)], axis=0)
        score = score + (sums.sum(axis=0) + tot[0] * 0) * 1e-7
        score = np.where(cnt < BLK, score, big)
        j = int(np.argmin(score))
        assign[i] = j
        cnt[j] += 1
        for d in range(nd):
            sums[d, j] += cnts[d][i]
    # repair per dim
    members = {j: [i for i, jj in assign.items() if jj == j] for j in range(nb)}
    for d in range(nd):
        for _ in range(2000):
            j = int(np.argmax(sums[d]))
            if sums[d, j] <= cap:
                break
            ms = members[j]
            pos_m = [i for i in ms if cnts[d][i] > 0]
            if not pos_m:
                break
            mv = min(pos_m, key=lambda i: cnts[d][i])
            tgt = np.where(cnt < BLK, sums[d], big)
            tgt[j] = big
            jt = int(np.argmin(tgt))
            if tgt[jt] >= big:
                break
            assign[mv] = jt
            members[j].remove(mv)
            members[jt].append(mv)
            cnt[j] -= 1
            cnt[jt] += 1
            for dd in range(nd):
                sums[dd, j] -= cnts[dd][mv]
                sums[dd, jt] += cnts[dd][mv]
    return assign


def _positions_from_assign(assign, block_ids):
    pos = {}
    slot = {j: 0 for j in block_ids}
    for i in sorted(assign):
        j = assign[i]
        pos[i] = j * BLK + slot[j]
        slot[j] += 1
    return pos


def _pack_blocks(cntA, cntB, cap=1148):
    nodes = np.arange(RPC)
    assign = _pack_greedy(nodes, [cntA, cntB], list(range(BPC)), cap)
    posd = _positions_from_assign(assign, list(range(BPC)))
    pos = np.empty(RPC, dtype=np.int64)
    for i in range(RPC):
        pos[i] = posd[i]
    return pos


def _pack_blocks4(cntA, cntB, cntC, cntD, half0_nodes, cap=1148):
    """Second pass: rebalance within halves on 4 dims."""
    pos = np.empty(RPC, dtype=np.int64)
    all_nodes = np.arange(RPC)
    h0 = half0_nodes
    h1 = all_nodes[~np.isin(all_nodes, h0)]
    for nodes, blocks in ((h0, list(range(S0_BLOCKS))),
                          (h1, list(range(S0_BLOCKS, BPC)))):
        assign = _pack_greedy(nodes, [cntA, cntB, cntC, cntD], blocks, cap)
        posd = {}
        slot = {j: 0 for j in range(len(blocks))}
        for i in sorted(assign):
            j = assign[i]
            posd[i] = blocks[j] * BLK + slot[j]
            slot[j] += 1
        for i in nodes:
            pos[i] = posd[i]
    return pos


def _build_stream(e_pos, e_idx16, K, e_par=None):
    """Returns (idx_wrapped [128, SL/16] i16, pos... ).
    pos[p, ci] = dst position within block for slot ci*128+p, 255 for pads.
    With e_par (0/1 per edge): returns parity-split (pos_even, pos_odd)."""
    nch = BPC * K
    SL = nch * BLK
    blk = e_pos // BLK
    o = np.argsort(blk, kind="stable")
    blk_s = blk[o]
    e_pos = e_pos[o]
    e_idx16 = e_idx16[o]
    if e_par is not None:
        e_par = e_par[o]
    counts = np.bincount(blk_s, minlength=BPC)
    assert counts.max() <= K * BLK, (counts.max(), K * BLK)
    starts = np.concatenate([[0], np.cumsum(counts)[:-1]])
    within = np.arange(len(blk_s)) - np.repeat(starts, counts)
    slot = blk_s * (K * BLK) + within

    idx_full = np.zeros(SL, dtype=np.int16)
    idx_full[slot] = e_idx16
    idx_w = np.tile(idx_full.reshape(SL // 16, 16).T, (8, 1)).copy()

    if e_par is None:
        pos_full = np.full(SL, 255, dtype=np.int16)
        pos_full[slot] = (e_pos % BLK).astype(np.int16)
        pos = pos_full.reshape(nch, BLK).T.copy()
        return idx_w, pos
    pe = np.full(SL, 255, dtype=np.int16)
    po = np.full(SL, 255, dtype=np.int16)
    pv = (e_pos % BLK).astype(np.int16)
    pe[slot[e_par == 0]] = pv[e_par == 0]
    po[slot[e_par == 1]] = pv[e_par == 1]
    return (idx_w, pe.reshape(nch, BLK).T.copy(),
            po.reshape(nch, BLK).T.copy())


def _preprocess_structure(edge_index):
    """Graph-structure-only preprocessing (cacheable)."""
    src = np.asarray(edge_index[0], dtype=np.int64)
    dst = np.asarray(edge_index[1], dtype=np.int64)
    loops = np.arange(N_NODES, dtype=np.int64)
    src_all = np.concatenate([src, loops])
    dst_all = np.concatenate([dst, loops])
    deg = np.bincount(dst_all, minlength=N_NODES).astype(np.float32)
    dinv = np.where(deg > 0, 1.0 / np.sqrt(deg), 0.0).astype(np.float32)

    core_of = dst_all // RPC

    perms = []
    core_edges = []
    cnts_ab = []
    for c in range(NCORES):
        m = core_of == c
        s_c = src_all[m]
        d_loc = (dst_all[m] - c * RPC).astype(np.int64)
        cntA = np.bincount(d_loc[s_c < SPLIT1], minlength=RPC)
        cntB = np.bincount(d_loc[s_c >= SPLIT1], minlength=RPC)
        perms.append(_pack_blocks(cntA, cntB))
        core_edges.append((s_c, d_loc))
        cnts_ab.append((cntA, cntB))

    permpos_global = np.empty(N_NODES, dtype=np.int64)
    for c in range(NCORES):
        permpos_global[c * RPC:(c + 1) * RPC] = perms[c]

    # pass 2: rebalance within halves, also evening C/D (src-half) counts
    half_global = permpos_global < S0_ROWS
    perms2 = []
    for c in range(NCORES):
        s_c, d_loc = core_edges[c]
        cntA, cntB = cnts_ab[c]
        hsrc = half_global[s_c]
        cntC = np.bincount(d_loc[hsrc], minlength=RPC)
        cntD = np.bincount(d_loc[~hsrc], minlength=RPC)
        half0_nodes = np.where(perms[c] < S0_ROWS)[0]
        perms2.append(_pack_blocks4(cntA, cntB, cntC, cntD, half0_nodes))
    perms = perms2
    for c in range(NCORES):
        permpos_global[c * RPC:(c + 1) * RPC] = perms[c]

    def seg_K(e_pos):
        counts = np.bincount(e_pos // BLK, minlength=BPC)
        return int(np.ceil(counts.max() / BLK))

    K1A = K1B = K2C = K2D = 1
    meta = []
    for c in range(NCORES):
        s_c, d_loc = core_edges[c]
        pos_d = perms[c][d_loc]
        mA = s_c < SPLIT1
        src_r = s_c // RPC
        src_pos = permpos_global[s_c]   # core-local position (0..RPAD-1)
        mC = src_pos < S0_ROWS
        K1A = max(K1A, seg_K(pos_d[mA]))
        K1B = max(K1B, seg_K(pos_d[~mA]))
        K2C = max(K2C, seg_K(pos_d[mC]))
        K2D = max(K2D, seg_K(pos_d[~mC]))
        meta.append((s_c, d_loc, pos_d, mA, mC, src_r, src_pos))

    streams = []
    for c in range(NCORES):
        s_c, d_loc, pos_d, mA, mC, src_r, src_pos = meta[c]
        i1a, p1a = _build_stream(pos_d[mA], s_c[mA].astype(np.int16), K1A)
        i1b, p1b = _build_stream(pos_d[~mA],
                                 (s_c[~mA] - SPLIT1).astype(np.int16), K1B)
        # phase-2 rows gathered in PAIRS (256B descriptors): idx = row//2,
        # the row parity selects the 64-col half of the gathered 128 cols.
        idxC = (src_r * S0_ROWS + src_pos).astype(np.int64)
        idxD = (src_r * S1_ROWS + (src_pos - S0_ROWS)).astype(np.int64)
        i2c, p2ce, p2co = _build_stream(
            pos_d[mC], (idxC[mC] // 2).astype(np.int16), K2C,
            e_par=(idxC[mC] % 2))
        i2d, p2de, p2do = _build_stream(
            pos_d[~mC], (idxD[~mC] // 2).astype(np.int16), K2D,
            e_par=(idxD[~mC] % 2))

        dinvb = np.zeros((BLK, BPC), dtype=np.float32)
        nodes_at = np.full(RPAD, -1, dtype=np.int64)
        nodes_at[perms[c]] = np.arange(RPC)
        valid = nodes_at >= 0
        dv = np.zeros(RPAD, np.float32)
        dv[valid] = dinv[nodes_at[valid] + c * RPC]
        dinvb[:, :] = dv.reshape(BPC, BLK).T

        streams.append({
            "i1a": i1a, "p1a": p1a, "i1b": i1b, "p1b": p1b,
            "i2c": i2c, "p2ce": p2ce, "p2co": p2co,
            "i2d": i2d, "p2de": p2de, "p2do": p2do,
            "dinvb": dinvb,
        })
    permarr = np.stack(perms)
    return {
        "streams": streams, "perms": permarr, "dinv": dinv,
        "Ks": np.array([K1A, K1B, K2C, K2D]),
    }


def _cached_structure(edge_index):
    ei = np.ascontiguousarray(np.asarray(edge_index, dtype=np.int64))
    h = hashlib.md5(ei.tobytes()).hexdigest()[:16]
    path = f"/tmp/gcn_pre_{h}_v4.npz"
    try:
        if os.path.exists(path):
            z = np.load(path)
            streams = []
            for c in range(NCORES):
                streams.append({k: z[f"{k}_{c}"] for k in
                                ("i1a", "p1a", "i1b", "p1b",
                                 "i2c", "p2ce", "p2co",
                                 "i2d", "p2de", "p2do", "dinvb")})
            return {"streams": streams, "perms": z["perms"],
                    "dinv": z["dinv"], "Ks": z["Ks"]}
    except Exception:
        pass
    st = _preprocess_structure(edge_index)
    try:
        flat = {"perms": st["perms"], "dinv": st["dinv"], "Ks": st["Ks"]}
        for c in range(NCORES):
            for k, v in st["streams"][c].items():
                flat[f"{k}_{c}"] = v
        np.savez(path + ".tmp.npz", **flat)
        os.replace(path + ".tmp.npz", path)
    except Exception:
        pass
    return st


def _preprocess(x, edge_index, W1, b1, W2, b2):
    st = _cached_structure(edge_index)
    dinv = st["dinv"]
    K1A, K1B, K2C, K2D = (int(v) for v in st["Ks"])

    xs = (np.asarray(x, np.float32) * dinv[:, None])
    GRP = 16
    NGRP = (NBLOCKS + GRP - 1) // GRP      # 25
    NPAD2 = NGRP * GRP * BLK               # 51200
    xT = np.zeros((D_IN, NPAD2), dtype=FP8)
    xT[:, :N_NODES] = xs.T.astype(FP8)
    # pre-tiled contiguous layout: [group, partition, kchunk, cols]
    xtg = np.ascontiguousarray(
        xT.reshape(4, 128, NGRP, GRP * BLK).transpose(2, 1, 0, 3))
    w1q = np.asarray(W1, np.float32).astype(FP8)
    b1rep = np.tile(np.asarray(b1, np.float32)[None, :], (128, 1)).copy()
    ident = np.eye(128, dtype=np.float32).astype(BF16)
    b2rep = np.tile(np.asarray(b2, np.float32)[None, :], (128, 1)).copy()
    w2b = np.asarray(W2, np.float32).astype(BF16)

    in_maps = []
    for c in range(NCORES):
        s = st["streams"][c]
        in_maps.append({
            "xTg": xtg, "w1": w1q, "w2": w2b, "b1rep": b1rep, "ident": ident,
            "b2rep": b2rep, "dinvb": np.ascontiguousarray(s["dinvb"]),
            "i1a": s["i1a"], "p1a": s["p1a"],
            "i1b": s["i1b"], "p1b": s["p1b"],
            "i2c": s["i2c"], "p2ce": s["p2ce"], "p2co": s["p2co"],
            "i2d": s["i2d"], "p2de": s["p2de"], "p2do": s["p2do"],
        })

    real = 800000 + N_NODES
    padded = NCORES * BLK * BPC * (K1A + K1B)
    LAST["K"] = (K1A, K1B, K2C, K2D)
    LAST["pad_frac"] = padded / real - 1.0
    perms = [st["perms"][c] for c in range(NCORES)]
    return in_maps, perms, (K1A, K1B, K2C, K2D)


# ---------------- device program ----------------

def _build_program(K1A, K1B, K2C, K2D):
    dt = mybir.dt
    DR = mybir.MatmulPerfMode.DoubleRow
    phases = int(os.environ.get("GCN_PHASES", "3"))
    nc = bacc.Bacc(None, target_bir_lowering=False, debug=False,
                   num_devices=NCORES, num_swdge_queues=NQ)

    GRP = 16
    NGRP = (NBLOCKS + GRP - 1) // GRP
    xTg = nc.dram_tensor("xTg", [NGRP, 128, 4, GRP * BLK], dt.float8e4,
                         kind="ExternalInput")
    w1 = nc.dram_tensor("w1", [D_IN, D_HID], dt.float8e4, kind="ExternalInput")
    w2 = nc.dram_tensor("w2", [D_HID, D_OUT], dt.bfloat16, kind="ExternalInput")
    b1rep = nc.dram_tensor("b1rep", [128, D_HID], dt.float32, kind="ExternalInput")
    ident = nc.dram_tensor("ident", [128, 128], dt.bfloat16, kind="ExternalInput")
    b2rep = nc.dram_tensor("b2rep", [128, D_OUT], dt.float32, kind="ExternalInput")
    dinvb = nc.dram_tensor("dinvb", [128, BPC], dt.float32, kind="ExternalInput")

    def idx_t(name, K):
        return nc.dram_tensor(name, [128, BPC * K * BLK // 16], dt.int16,
                              kind="ExternalInput")

    def pos_t(name, K):
        return nc.dram_tensor(name, [128, BPC * K], dt.int16,
                              kind="ExternalInput")

    i1a, p1a = idx_t("i1a", K1A), pos_t("p1a", K1A)
    i1b, p1b = idx_t("i1b", K1B), pos_t("p1b", K1B)
    i2c = idx_t("i2c", K2C)
    p2ce, p2co = pos_t("p2ce", K2C), pos_t("p2co", K2C)
    i2d = idx_t("i2d", K2D)
    p2de, p2do = pos_t("p2de", K2D), pos_t("p2do", K2D)

    out = nc.dram_tensor("out", [RPAD, D_OUT], dt.float32, kind="ExternalOutput")

    z1A = nc.dram_tensor("z1A", [SPLIT1, D_HID], dt.float8e4)
    z1B = nc.dram_tensor("z1B", [NPAD - SPLIT1, D_HID], dt.float8e4)
    # z2 rows unpadded (64 cols bf16 = 128B); the gather fetches PAIRS of
    # rows per 256B descriptor via a [rows/2, 128] view of these tensors.
    z2in0 = nc.dram_tensor("z2in0", [S0_ROWS, D_OUT], dt.bfloat16)
    z2in1 = nc.dram_tensor("z2in1", [S1_ROWS, D_OUT], dt.bfloat16)
    z2out0 = nc.dram_tensor("z2out0", [NCORES * S0_ROWS, D_OUT], dt.bfloat16,
                            addr_space="Shared")
    z2out1 = nc.dram_tensor("z2out1", [NCORES * S1_ROWS, D_OUT], dt.bfloat16,
                            addr_space="Shared")

    qctr = [0]

    def next_q():
        q = qctr[0] % NQ
        qctr[0] += 1
        return q

    with tile.TileContext(nc) as tc:
        with tc.tile_pool(name="consts", bufs=1) as cp, \
             tc.tile_pool(name="ph0x", bufs=2) as xp, \
             tc.tile_pool(name="ph0o", bufs=3) as op0, \
             tc.tile_pool(name="gp", bufs=8) as gp, \
             tc.tile_pool(name="gp2", bufs=8) as gp2, \
             tc.tile_pool(name="ohp", bufs=3) as ohp, \
             tc.tile_pool(name="csp", bufs=BPC + 1) as csp, \
             tc.tile_pool(name="hp", bufs=2) as hp, \
             tc.tile_pool(name="zp", bufs=3) as zp, \
             tc.tile_pool(name="smp", bufs=8) as smp, \
             tc.tile_pool(name="psAcc", bufs=3, space="PSUM") as psAcc, \
             tc.tile_pool(name="psMisc", bufs=1, space="PSUM") as psMisc, \
             tc.tile_pool(name="psO", bufs=3, space="PSUM") as psO:
            w1t = cp.tile([128, 4, D_HID], dt.float8e4)
            nc.sync.dma_start(
                w1t[:], w1.ap().rearrange("(k p) n -> p k n", p=128))
            w2t = cp.tile([128, 2, D_OUT], dt.bfloat16)
            nc.sync.dma_start(
                w2t[:], w2.ap().rearrange("(k p) n -> p k n", p=128))
            b1r = cp.tile([128, D_HID], dt.float32)
            nc.sync.dma_start(b1r[:], b1rep[:, :])
            idt = cp.tile([128, 128], dt.bfloat16)
            nc.sync.dma_start(idt[:], ident[:, :])
            b2t = cp.tile([128, D_OUT], dt.float32)
            nc.sync.dma_start(b2t[:], b2rep[:, :])
            dvt = cp.tile([128, BPC], dt.float32)
            nc.sync.dma_start(dvt[:], dinvb[:, :])
            it1a = cp.tile([128, BPC * K1A * BLK // 16], dt.int16)
            nc.scalar.dma_start(it1a[:], i1a[:, :])
            it1b = cp.tile([128, BPC * K1B * BLK // 16], dt.int16)
            nc.scalar.dma_start(it1b[:], i1b[:, :])
            it2c = cp.tile([128, BPC * K2C * BLK // 16], dt.int16)
            nc.scalar.dma_start(it2c[:], i2c[:, :])
            it2d = cp.tile([128, BPC * K2D * BLK // 16], dt.int16)
            nc.scalar.dma_start(it2d[:], i2d[:, :])
            pt1a = cp.tile([128, BPC * K1A], dt.int16)
            nc.scalar.dma_start(pt1a[:], p1a[:, :])
            pt1b = cp.tile([128, BPC * K1B], dt.int16)
            nc.scalar.dma_start(pt1b[:], p1b[:, :])
            pt2ce = cp.tile([128, BPC * K2C], dt.int16)
            nc.scalar.dma_start(pt2ce[:], p2ce[:, :])
            pt2co = cp.tile([128, BPC * K2C], dt.int16)
            nc.scalar.dma_start(pt2co[:], p2co[:, :])
            pt2de = cp.tile([128, BPC * K2D], dt.int16)
            nc.scalar.dma_start(pt2de[:], p2de[:, :])
            pt2do = cp.tile([128, BPC * K2D], dt.int16)
            nc.scalar.dma_start(pt2do[:], p2do[:, :])
            iot = cp.tile([128, 128], dt.int16)
            nc.gpsimd.iota(iot[:], [[1, 128]], base=0, channel_multiplier=0)
            iot3 = iot[:].rearrange("p (o j) -> p o j", o=1)

            # ------- phase 0: z1 = xT^T @ W1, fp8 DoubleRow -------
            z1Av = z1A.ap().rearrange("(n p) f -> p n f", p=128)
            z1Bv = z1B.ap().rearrange("(n p) f -> p n f", p=128)
            NB_A = SPLIT1 // BLK
            GB = 7
            for g0 in range(0, NBLOCKS, GRP):
                gb = min(GRP, NBLOCKS - g0)
                xg = xp.tile([128, 4, GRP * BLK], dt.float8e4, tag="xg")
                nc.sync.dma_start(
                    xg[:, :, :gb * BLK],
                    xTg.ap()[g0 // GRP, :, :, :gb * BLK])
                for b0 in range(0, gb, GB):
                    nb = min(GB, gb - b0)
                    zo = op0.tile([128, GB, D_HID], dt.float8e4, tag="zo")
                    for i in range(nb):
                        ps = psAcc.tile([128, D_HID], dt.float32, tag="acc")
                        col = (b0 + i) * BLK
                        for t in range(2):
                            nc.tensor.matmul(
                                ps[:],
                                xg[:, 2 * t:2 * t + 2, col:col + BLK],
                                w1t[:, 2 * t:2 * t + 2, :],
                                start=(t == 0), stop=(t == 1),
                                perf_mode=DR)
                        nc.vector.tensor_copy(zo[:, i, :], ps[:])
                    lo, hi = g0 + b0, g0 + b0 + nb
                    if hi <= NB_A:
                        nc.sync.dma_start(z1Av[:, lo:hi, :], zo[:, :nb, :])
                    elif lo >= NB_A:
                        nc.sync.dma_start(
                            z1Bv[:, lo - NB_A:hi - NB_A, :], zo[:, :nb, :])
                    else:
                        na = NB_A - lo
                        nc.sync.dma_start(z1Av[:, lo:NB_A, :], zo[:, :na, :])
                        nc.sync.dma_start(
                            z1Bv[:, 0:hi - NB_A, :], zo[:, na:nb, :])

            # ---------------- phases 1+2 ----------------
            seg1 = {
                "A": (K1A, it1a, pt1a, z1A.ap()[:, :]),
                "B": (K1B, it1b, pt1b, z1B.ap()[:, :]),
            }
            seg2 = {
                "C": (K2C, it2c, (pt2ce, pt2co),
                      z2out0.ap().rearrange("(r two) f -> r (two f)", two=2)),
                "D": (K2D, it2d, (pt2de, pt2do),
                      z2out1.ap().rearrange("(r two) f -> r (two f)", two=2)),
            }
            gtiles = {}

            def ensure_g(layer, s, pi):
                key = (layer, s, pi)
                if key in gtiles:
                    return gtiles[key]
                K, itile, _, zview = (seg1 if layer == 1 else seg2)[s]
                felem = D_HID if layer == 1 else 2 * D_OUT
                fdt = dt.float8e4 if layer == 1 else dt.bfloat16
                SL = BPC * K * BLK
                n = min(PIECE, SL - pi * PIECE)
                off = pi * (PIECE // 16)
                pool = gp if layer == 1 else gp2
                gt = pool.tile([128, CHPP, felem], fdt, tag=f"g{layer}{s}")
                nc.gpsimd.dma_gather(
                    gt[:, :n // 128, :], zview, itile[:, off:off + n // 16],
                    n, n, felem, queue_num=next_q())
                gtiles[key] = gt
                return gt

            def make_onehot_t(ptile, K, b, odt, tag):
                oh = ohp.tile([128, K, 128], odt, tag=tag)
                pv = ptile[:, b * K:(b + 1) * K].rearrange(
                    "p (c o) -> p c o", o=1)
                a0, a1 = broadcast_tensor_aps(iot3, pv)
                nc.vector.tensor_tensor(
                    oh[:], a0, a1, op=mybir.AluOpType.is_equal)
                return oh

            def make_onehot(layer, s, b):
                K, _, ptile, _ = seg1[s]
                return make_onehot_t(ptile, K, b, dt.float8e4, f"oh{layer}{s}")

            def l1_block(b):
                hps = psAcc.tile([128, D_HID], dt.float32, tag="acc")
                first = True
                for s in ("A", "B"):
                    K = seg1[s][0]
                    oh = make_onehot(1, s, b)
                    k = 0
                    while k < K:
                        ci = b * K + k
                        pi, cpos = divmod(ci, CHPP)
                        last_s = (s == "B")
                        gt = ensure_g(1, s, pi)
                        if k + 1 < K and cpos + 1 < CHPP:
                            nc.tensor.matmul(
                                hps[:], oh[:, k:k + 2, :],
                                gt[:, cpos:cpos + 2, :],
                                start=first, stop=(last_s and k + 2 == K),
                                perf_mode=DR)
                            k += 2
                        else:
                            nc.tensor.matmul(
                                hps[:], oh[:, k, :], gt[:, cpos, :],
                                start=first, stop=(last_s and k + 1 == K))
                            k += 1
                        first = False
                hs = hp.tile([128, D_HID], dt.float32, tag="hs")
                nc.vector.tensor_scalar(
                    hs[:], hps[:], dvt[:, b:b + 1], None,
                    op0=mybir.AluOpType.mult)
                hb = hp.tile([128, D_HID], dt.bfloat16, tag="hb")
                nc.vector.tensor_tensor(
                    hb[:], hs[:], b1r[:], op=mybir.AluOpType.add)
                hr = hp.tile([128, D_HID], dt.bfloat16, tag="hr")
                nc.scalar.activation(
                    hr[:], hb[:], mybir.ActivationFunctionType.Relu)
                hT = hp.tile([128, 2, 128], dt.bfloat16, tag="hT")
                for h in range(2):
                    tps = psMisc.tile([128, 128], dt.bfloat16, tag="tps")
                    nc.tensor.transpose(
                        tps[:], hr[:, h * 128:(h + 1) * 128], idt[:])
                    nc.scalar.copy(hT[:, h, :], tps[:])
                zps = psMisc.tile([128, D_OUT], dt.float32, tag="zps")
                for h in range(2):
                    nc.tensor.matmul(
                        zps[:], hT[:, h, :], w2t[:, h, :],
                        start=(h == 0), stop=(h == 1))
                z2s = zp.tile([128, D_OUT], dt.bfloat16, tag="z2s")
                nc.vector.tensor_scalar(
                    z2s[:], zps[:], dvt[:, b:b + 1], None,
                    op0=mybir.AluOpType.mult)
                if b < S0_BLOCKS:
                    nc.sync.dma_start(
                        z2in0.ap()[b * BLK:(b + 1) * BLK, :], z2s[:])
                else:
                    bb = b - S0_BLOCKS
                    nc.sync.dma_start(
                        z2in1.ap()[bb * BLK:(bb + 1) * BLK, :], z2s[:])

            cstash = {}

            def l2cd_block(b, s):
                ops = psO.tile([128, D_OUT], dt.float32, tag="ops")
                K, _, (pte, pto), _ = seg2[s]
                ohe = make_onehot_t(pte, K, b, dt.bfloat16, f"oh2{s}e")
                oho = make_onehot_t(pto, K, b, dt.bfloat16, f"oh2{s}o")
                for k in range(K):
                    ci = b * K + k
                    pi, cpos = divmod(ci, CHPP)
                    gt = ensure_g(2, s, pi)
                    nc.tensor.matmul(
                        ops[:], ohe[:, k, :], gt[:, cpos, :D_OUT],
                        start=(k == 0), stop=False)
                    nc.tensor.matmul(
                        ops[:], oho[:, k, :], gt[:, cpos, D_OUT:],
                        start=False, stop=(k == K - 1))
                return ops

            def l2c_block(b):
                ops = l2cd_block(b, "C")
                cs0 = zp.tile([128, D_OUT], dt.float32, tag="cs0")
                nc.scalar.activation(
                    cs0[:], ops[:], mybir.ActivationFunctionType.Copy,
                    scale=dvt[:, b:b + 1])
                cs = csp.tile([128, D_OUT], dt.float32, tag="cs")
                nc.vector.tensor_tensor(
                    cs[:], cs0[:], b2t[:], op=mybir.AluOpType.add)
                cstash[b] = cs

            def l2d_block(b):
                ops = l2cd_block(b, "D")
                t = smp.tile([128, D_OUT], dt.float32, tag="t")
                nc.vector.tensor_scalar(
                    t[:], ops[:], dvt[:, b:b + 1], None,
                    op0=mybir.AluOpType.mult)
                t2 = smp.tile([128, D_OUT], dt.float32, tag="t2")
                nc.vector.tensor_tensor(
                    t2[:], t[:], cstash[b][:], op=mybir.AluOpType.add)
                nm = smp.tile([128, 1], dt.float32, tag="nm")
                nc.vector.reduce_max(
                    nm[:], t2[:], axis=mybir.AxisListType.X, negate=True)
                ex = smp.tile([128, D_OUT], dt.float32, tag="ex")
                sm = smp.tile([128, 1], dt.float32, tag="sm")
                nc.scalar.activation(
                    ex[:], t2[:], mybir.ActivationFunctionType.Exp,
                    bias=nm[:], accum_out=sm[:])
                rc = smp.tile([128, 1], dt.float32, tag="rc")
                nc.vector.reciprocal(rc[:], sm[:])
                ot = smp.tile([128, D_OUT], dt.float32, tag="ot")
                nc.vector.tensor_scalar(
                    ot[:], ex[:], rc[:], None, op0=mybir.AluOpType.mult)
                nc.sync.dma_start(out.ap()[b * BLK:(b + 1) * BLK, :], ot[:])

            if phases >= 1:
                for b in range(32):
                    l1_block(b)
                if phases >= 2:
                    nc.gpsimd.collective_compute(
                        "AllGather", mybir.AluOpType.bypass,
                        replica_groups=[list(range(NCORES))],
                        ins=[z2in0.ap().opt()], outs=[z2out0.ap().opt()])
                ci = 0
                for b in range(32, BPC):
                    l1_block(b)
                    if phases >= 3 and b >= 40 and ci < BPC:
                        l2c_block(ci)
                        ci += 1
                if phases >= 2:
                    nc.gpsimd.collective_compute(
                        "AllGather", mybir.AluOpType.bypass,
                        replica_groups=[list(range(NCORES))],
                        ins=[z2in1.ap().opt()], outs=[z2out1.ap().opt()])
                if phases >= 3:
                    while ci < BPC:
                        l2c_block(ci)
                        ci += 1
                    for b in range(BPC):
                        l2d_block(b)

    nc.compile()

    # The Tile scheduler reorders instructions; DMASW sem lanes rotate mod 8
    # over Pool-engine DMAs in FINAL order and each sem may only be driven by
    # one SWDGE queue.  Rewrite queue_num to lane%NQ in final order so the
    # lane<->queue pairing is always consistent.
    if os.environ.get("GCN_QFIX", "1") == "1":
        from concourse.tile_scheduler import DMAInst
        idx = 0
        for blk in nc.m.functions[0].blocks:
            for inst in blk.instructions:
                if (inst.engine == mybir.EngineType.Pool
                        and isinstance(inst, DMAInst)):
                    inst.queue_num = (idx % 8) % NQ
                    idx += 1
    return nc


# ---------------- entry point ----------------

def kernel(x, edge_index, W1, b1, W2, b2):
    x = np.asarray(x)
    edge_index = np.asarray(edge_index)
    in_maps, perms, Ks = _preprocess(x, edge_index, W1, b1, W2, b2)
    nc = _build_program(*Ks)

    trace = os.environ.get("GCN_TRACE", "0") == "1"
    if trace:
        trace = _install_trace_hook()
    res = run_bass_kernel_spmd(
        nc, in_maps, core_ids=list(range(NCORES)), trace=trace)
    LAST["exec_time_ns"] = res.exec_time_ns
    LAST["results"] = res

    out = np.empty((N_NODES, D_OUT), dtype=np.float32)
    for c in range(NCORES):
        oc = np.asarray(res.results[c]["out"], dtype=np.float32)
        out[c * RPC:(c + 1) * RPC] = oc[perms[c]]
    return out


# revision 37
# speedup vs baseline: 1.1526x; 1.0200x over previous
"""GCN (2-layer, PyG GCNConv semantics) on 8 Trainium2 NeuronCores.

V2 strategy (dst-sharded message passing, fp8 front end):
  out = softmax( A @ relu(A @ (x W1) + b1) @ W2 + b2 ),  A = D^-1/2 (Adj+I) D^-1/2

  - Host: degrees/dinv, self-loops appended as ordinary edges, edges
    partitioned by destination core (6250 dst rows per core), each core's
    dst nodes permuted into 50 load-balanced blocks of 128.  Per-edge
    gather indices (int16, SWDGE wrapped) and compact per-slot dst-position
    streams (int16) are precomputed on host (cached to /tmp keyed on the
    edge list).
  - Phase 0 (on-device, redundant per core): z1 = (dinv*x) @ W1 computed
    in fp8 with DoubleRow perf mode, stored fp8 to local HBM.
  - Phase 1: per-edge dma_gather of fp8 z1 rows (4 SWDGE queues);
    one-hot matrices generated ON CHIP (iota + is_equal, DVE) instead of
    streamed; segment-sum via fp8 DoubleRow TensorE matmuls; bias+relu;
    z2 = dinv * (h @ W2) in bf16 per block.
  - AllGather of bf16 z2 across the 8 cores in two row-slices,
    overlapped with phase-1/2 compute.
  - Phase 2: per-edge dma_gather of bf16 z2 rows, on-chip one-hot
    segment-sum, + b2, softmax, DMA out.

kernel(**inputs) -> np.ndarray is self-contained (shapes hardcoded).
"""

import hashlib
import os
import sys
import types

sys.path.insert(0, "/opt/trn_rl_repo")

import numpy as np
import ml_dtypes

from concourse import bass, mybir, bacc, tile
from concourse.bass import broadcast_tensor_aps
from concourse.bass_utils import run_bass_kernel_spmd

BF16 = ml_dtypes.bfloat16
FP8 = ml_dtypes.float8_e4m3

# ---------------- problem constants (hardcoded) ----------------
N_NODES = 50000
D_IN, D_HID, D_OUT = 512, 256, 64
NCORES = 8
RPC = N_NODES // NCORES          # 6250 dst rows per core
BLK = 128
BPC = 50                         # blocks per core (spare slots for balancing)
RPAD = BPC * BLK                 # 6400
NPAD = ((N_NODES + BLK - 1) // BLK) * BLK   # 50048 (391 node blocks)
NBLOCKS = NPAD // BLK            # 391
SPLIT1 = 24960                   # L1 gather src split (block-aligned, int16-safe)
S0_ROWS = 3200                   # AG slice 0: perm positions [0, 3200) = 25 blocks
S1_ROWS = RPAD - S0_ROWS         # 3200: positions [3200, 6400) = 25 blocks
S0_BLOCKS = S0_ROWS // BLK       # 25
PIECE = 512                      # gather slots per dma_gather call (ring-safe)
CHPP = PIECE // BLK              # chunks per piece (4)
NQ = 4                           # SWDGE queues (ucode max)

LAST = {}                        # test harness introspection


def _install_trace_hook():
    try:
        mod = types.ModuleType("antenv.axon_hooks")
        hook = [None]
        mod.set_axon_ntff_profile_hook = lambda h: hook.__setitem__(0, h)
        mod.get_axon_ntff_profile_hook = lambda: hook[0]
        sys.modules["antenv.axon_hooks"] = mod
        import antenv
        antenv.axon_hooks = mod
        from trn_agent_boot.trn_boot import _ntff_profile_via_ctypes
        mod.set_axon_ntff_profile_hook(
            _ntff_profile_via_ctypes("/opt/axon/libaxon_pjrt.so"))
        return True
    except Exception:
        return False


# ---------------- host-side preprocessing ----------------

def _pack_greedy(node_ids, cnts, block_ids, cap):
    """Greedy k-dim balanced packing of node_ids into block_ids (<=128 each).
    cnts: [ndim, RPC] per-node counts. Returns {node: block}."""
    nd = len(cnts)
    nb = len(block_ids)
    tot = sum(c[node_ids] for c in cnts)
    order = node_ids[np.argsort(-tot, kind="stable")]
    sums = np.zeros((nd, nb), dtype=np.float64)
    cnt = np.zeros(nb, dtype=np.int64)
    assign = {}
    big = 1e18
    for i in order:
        score = np.max([(sums[d] + cnts[d][i]) / cap for d in range(nd)], axis=0)
        score = score + (sums.sum(axis=0) + tot[0] * 0) * 1e-7
        score = np.where(cnt < BLK, score, big)
        j = int(np.argmin(score))
        assign[i] = j
        cnt[j] += 1
        for d in range(nd):
            sums[d, j] += cnts[d][i]
    # repair per dim
    members = {j: [i for i, jj in assign.items() if jj == j] for j in range(nb)}
    for d in range(nd):
        for _ in range(2000):
            j = int(np.argmax(sums[d]))
            if sums[d, j] <= cap:
                break
            ms = members[j]
            pos_m = [i for i in ms if cnts[d][i] > 0]
            if not pos_m:
                break
            mv = min(pos_m, key=lambda i: cnts[d][i])
            tgt = np.where(cnt < BLK, sums[d], big)
            tgt[j] = big
            jt = int(np.argmin(tgt))
            if tgt[jt] >= big:
                break
            assign[mv] = jt
            members[j].remove(mv)
            members[jt].append(mv)
            cnt[j] -= 1
            cnt[jt] += 1
            for dd in range(nd):
                sums[dd, j] -= cnts[dd][mv]
                sums[dd, jt] += cnts[dd][mv]
    return assign


def _positions_from_assign(assign, block_ids):
    pos = {}
    slot = {j: 0 for j in block_ids}
    for i in sorted(assign):
        j = assign[i]
        pos[i] = j * BLK + slot[j]
        slot[j] += 1
    return pos


def _pack_blocks(cntA, cntB, cap=1148):
    nodes = np.arange(RPC)
    assign = _pack_greedy(nodes, [cntA, cntB], list(range(BPC)), cap)
    posd = _positions_from_assign(assign, list(range(BPC)))
    pos = np.empty(RPC, dtype=np.int64)
    for i in range(RPC):
        pos[i] = posd[i]
    return pos


def _pack_blocks4(cntA, cntB, cntC, cntD, half0_nodes, cap=1148):
    """Second pass: rebalance within halves on 4 dims."""
    pos = np.empty(RPC, dtype=np.int64)
    all_nodes = np.arange(RPC)
    h0 = half0_nodes
    h1 = all_nodes[~np.isin(all_nodes, h0)]
    for nodes, blocks in ((h0, list(range(S0_BLOCKS))),
                          (h1, list(range(S0_BLOCKS, BPC)))):
        assign = _pack_greedy(nodes, [cntA, cntB, cntC, cntD], blocks, cap)
        posd = {}
        slot = {j: 0 for j in range(len(blocks))}
        for i in sorted(assign):
            j = assign[i]
            posd[i] = blocks[j] * BLK + slot[j]
            slot[j] += 1
        for i in nodes:
            pos[i] = posd[i]
    return pos


def _build_stream(e_pos, e_idx16, K, e_par=None):
    """Returns (idx_wrapped [128, SL/16] i16, pos... ).
    pos[p, ci] = dst position within block for slot ci*128+p, 255 for pads.
    With e_par (0/1 per edge): returns parity-split (pos_even, pos_odd)."""
    nch = BPC * K
    SL = nch * BLK
    blk = e_pos // BLK
    o = np.argsort(blk, kind="stable")
    blk_s = blk[o]
    e_pos = e_pos[o]
    e_idx16 = e_idx16[o]
    if e_par is not None:
        e_par = e_par[o]
    counts = np.bincount(blk_s, minlength=BPC)
    assert counts.max() <= K * BLK, (counts.max(), K * BLK)
    starts = np.concatenate([[0], np.cumsum(counts)[:-1]])
    within = np.arange(len(blk_s)) - np.repeat(starts, counts)
    slot = blk_s * (K * BLK) + within

    idx_full = np.zeros(SL, dtype=np.int16)
    idx_full[slot] = e_idx16
    idx_w = np.tile(idx_full.reshape(SL // 16, 16).T, (8, 1)).copy()

    if e_par is None:
        pos_full = np.full(SL, 255, dtype=np.int16)
        pos_full[slot] = (e_pos % BLK).astype(np.int16)
        pos = pos_full.reshape(nch, BLK).T.copy()
        return idx_w, pos
    pe = np.full(SL, 255, dtype=np.int16)
    po = np.full(SL, 255, dtype=np.int16)
    pv = (e_pos % BLK).astype(np.int16)
    pe[slot[e_par == 0]] = pv[e_par == 0]
    po[slot[e_par == 1]] = pv[e_par == 1]
    return (idx_w, pe.reshape(nch, BLK).T.copy(),
            po.reshape(nch, BLK).T.copy())


def _preprocess_structure(edge_index):
    """Graph-structure-only preprocessing (cacheable)."""
    src = np.asarray(edge_index[0], dtype=np.int64)
    dst = np.asarray(edge_index[1], dtype=np.int64)
    loops = np.arange(N_NODES, dtype=np.int64)
    src_all = np.concatenate([src, loops])
    dst_all = np.concatenate([dst, loops])
    deg = np.bincount(dst_all, minlength=N_NODES).astype(np.float32)
    dinv = np.where(deg > 0, 1.0 / np.sqrt(deg), 0.0).astype(np.float32)

    core_of = dst_all // RPC

    perms = []
    core_edges = []
    cnts_ab = []
    for c in range(NCORES):
        m = core_of == c
        s_c = src_all[m]
        d_loc = (dst_all[m] - c * RPC).astype(np.int64)
        cntA = np.bincount(d_loc[s_c < SPLIT1], minlength=RPC)
        cntB = np.bincount(d_loc[s_c >= SPLIT1], minlength=RPC)
        perms.append(_pack_blocks(cntA, cntB))
        core_edges.append((s_c, d_loc))
        cnts_ab.append((cntA, cntB))

    permpos_global = np.empty(N_NODES, dtype=np.int64)
    for c in range(NCORES):
        permpos_global[c * RPC:(c + 1) * RPC] = perms[c]

    # pass 2: rebalance within halves, also evening C/D (src-half) counts
    half_global = permpos_global < S0_ROWS
    perms2 = []
    for c in range(NCORES):
        s_c, d_loc = core_edges[c]
        cntA, cntB = cnts_ab[c]
        hsrc = half_global[s_c]
        cntC = np.bincount(d_loc[hsrc], minlength=RPC)
        cntD = np.bincount(d_loc[~hsrc], minlength=RPC)
        half0_nodes = np.where(perms[c] < S0_ROWS)[0]
        perms2.append(_pack_blocks4(cntA, cntB, cntC, cntD, half0_nodes))
    perms = perms2
    for c in range(NCORES):
        permpos_global[c * RPC:(c + 1) * RPC] = perms[c]

    def seg_K(e_pos):
        counts = np.bincount(e_pos // BLK, minlength=BPC)
        return int(np.ceil(counts.max() / BLK))

    K1A = K1B = K2C = K2D = 1
    meta = []
    for c in range(NCORES):
        s_c, d_loc = core_edges[c]
        pos_d = perms[c][d_loc]
        mA = s_c < SPLIT1
        src_r = s_c // RPC
        src_pos = permpos_global[s_c]   # core-local position (0..RPAD-1)
        mC = src_pos < S0_ROWS
        K1A = max(K1A, seg_K(pos_d[mA]))
        K1B = max(K1B, seg_K(pos_d[~mA]))
        K2C = max(K2C, seg_K(pos_d[mC]))
        K2D = max(K2D, seg_K(pos_d[~mC]))
        meta.append((s_c, d_loc, pos_d, mA, mC, src_r, src_pos))

    streams = []
    for c in range(NCORES):
        s_c, d_loc, pos_d, mA, mC, src_r, src_pos = meta[c]
        i1a, p1a = _build_stream(pos_d[mA], s_c[mA].astype(np.int16), K1A)
        i1b, p1b = _build_stream(pos_d[~mA],
                                 (s_c[~mA] - SPLIT1).astype(np.int16), K1B)
        # phase-2 rows gathered in PAIRS (256B descriptors): idx = row//2,
        # the row parity selects the 64-col half of the gathered 128 cols.
        idxC = (src_r * S0_ROWS + src_pos).astype(np.int64)
        idxD = (src_r * S1_ROWS + (src_pos - S0_ROWS)).astype(np.int64)
        i2c, p2ce, p2co = _build_stream(
            pos_d[mC], (idxC[mC] // 2).astype(np.int16), K2C,
            e_par=(idxC[mC] % 2))
        i2d, p2de, p2do = _build_stream(
            pos_d[~mC], (idxD[~mC] // 2).astype(np.int16), K2D,
            e_par=(idxD[~mC] % 2))

        dinvb = np.zeros((BLK, BPC), dtype=np.float32)
        nodes_at = np.full(RPAD, -1, dtype=np.int64)
        nodes_at[perms[c]] = np.arange(RPC)
        valid = nodes_at >= 0
        dv = np.zeros(RPAD, np.float32)
        dv[valid] = dinv[nodes_at[valid] + c * RPC]
        dinvb[:, :] = dv.reshape(BPC, BLK).T

        streams.append({
            "i1a": i1a, "p1a": p1a, "i1b": i1b, "p1b": p1b,
            "i2c": i2c, "p2ce": p2ce, "p2co": p2co,
            "i2d": i2d, "p2de": p2de, "p2do": p2do,
            "dinvb": dinvb,
        })
    permarr = np.stack(perms)
    return {
        "streams": streams, "perms": permarr, "dinv": dinv,
        "Ks": np.array([K1A, K1B, K2C, K2D]),
    }


def _cached_structure(edge_index):
    ei = np.ascontiguousarray(np.asarray(edge_index, dtype=np.int64))
    h = hashlib.md5(ei.tobytes()).hexdigest()[:16]
    path = f"/tmp/gcn_pre_{h}_v4.npz"
    try:
        if os.path.exists(path):
            z = np.load(path)
            streams = []
            for c in range(NCORES):
                streams.append({k: z[f"{k}_{c}"] for k in
                                ("i1a", "p1a", "i1b", "p1b",
                                 "i2c", "p2ce", "p2co",
                                 "i2d", "p2de", "p2do", "dinvb")})
            return {"streams": streams, "perms": z["perms"],
                    "dinv": z["dinv"], "Ks": z["Ks"]}
    except Exception:
        pass
    st = _preprocess_structure(edge_index)
    try:
        flat = {"perms": st["perms"], "dinv": st["dinv"], "Ks": st["Ks"]}
        for c in range(NCORES):
            for k, v in st["streams"][c].items():
                flat[f"{k}_{c}"] = v
        np.savez(path + ".tmp.npz", **flat)
        os.replace(path + ".tmp.npz", path)
    except Exception:
        pass
    return st


def _preprocess(x, edge_index, W1, b1, W2, b2):
    st = _cached_structure(edge_index)
    dinv = st["dinv"]
    K1A, K1B, K2C, K2D = (int(v) for v in st["Ks"])

    xs = (np.asarray(x, np.float32) * dinv[:, None])
    GRP = 16
    NGRP = (NBLOCKS + GRP - 1) // GRP      # 25
    NPAD2 = NGRP * GRP * BLK               # 51200
    xT = np.zeros((D_IN, NPAD2), dtype=FP8)
    xT[:, :N_NODES] = xs.T.astype(FP8)
    # pre-tiled contiguous layout: [group, partition, kchunk, cols]
    xtg = np.ascontiguousarray(
        xT.reshape(4, 128, NGRP, GRP * BLK).transpose(2, 1, 0, 3))
    w1q = np.asarray(W1, np.float32).astype(FP8)
    b1rep = np.tile(np.asarray(b1, np.float32)[None, :], (128, 1)).copy()
    ident = np.eye(128, dtype=np.float32).astype(BF16)
    b2rep = np.tile(np.asarray(b2, np.float32)[None, :], (128, 1)).copy()
    w2b = np.asarray(W2, np.float32).astype(BF16)

    in_maps = []
    for c in range(NCORES):
        s = st["streams"][c]
        in_maps.append({
            "xTg": xtg, "w1": w1q, "w2": w2b, "b1rep": b1rep, "ident": ident,
            "b2rep": b2rep, "dinvb": np.ascontiguousarray(s["dinvb"]),
            "i1a": s["i1a"], "p1a": s["p1a"],
            "i1b": s["i1b"], "p1b": s["p1b"],
            "i2c": s["i2c"], "p2ce": s["p2ce"], "p2co": s["p2co"],
            "i2d": s["i2d"], "p2de": s["p2de"], "p2do": s["p2do"],
        })

    real = 800000 + N_NODES
    padded = NCORES * BLK * BPC * (K1A + K1B)
    LAST["K"] = (K1A, K1B, K2C, K2D)
    LAST["pad_frac"] = padded / real - 1.0
    perms = [st["perms"][c] for c in range(NCORES)]
    return in_maps, perms, (K1A, K1B, K2C, K2D)


# ---------------- device program ----------------

def _build_program(K1A, K1B, K2C, K2D):
    dt = mybir.dt
    DR = mybir.MatmulPerfMode.DoubleRow
    phases = int(os.environ.get("GCN_PHASES", "3"))
    nc = bacc.Bacc(None, target_bir_lowering=False, debug=False,
                   num_devices=NCORES, num_swdge_queues=NQ)

    GRP = 16
    NGRP = (NBLOCKS + GRP - 1) // GRP
    xTg = nc.dram_tensor("xTg", [NGRP, 128, 4, GRP * BLK], dt.float8e4,
                         kind="ExternalInput")
    w1 = nc.dram_tensor("w1", [D_IN, D_HID], dt.float8e4, kind="ExternalInput")
    w2 = nc.dram_tensor("w2", [D_HID, D_OUT], dt.bfloat16, kind="ExternalInput")
    b1rep = nc.dram_tensor("b1rep", [128, D_HID], dt.float32, kind="ExternalInput")
    ident = nc.dram_tensor("ident", [128, 128], dt.bfloat16, kind="ExternalInput")
    b2rep = nc.dram_tensor("b2rep", [128, D_OUT], dt.float32, kind="ExternalInput")
    dinvb = nc.dram_tensor("dinvb", [128, BPC], dt.float32, kind="ExternalInput")

    def idx_t(name, K):
        return nc.dram_tensor(name, [128, BPC * K * BLK // 16], dt.int16,
                              kind="ExternalInput")

    def pos_t(name, K):
        return nc.dram_tensor(name, [128, BPC * K], dt.int16,
                              kind="ExternalInput")

    i1a, p1a = idx_t("i1a", K1A), pos_t("p1a", K1A)
    i1b, p1b = idx_t("i1b", K1B), pos_t("p1b", K1B)
    i2c = idx_t("i2c", K2C)
    p2ce, p2co = pos_t("p2ce", K2C), pos_t("p2co", K2C)
    i2d = idx_t("i2d", K2D)
    p2de, p2do = pos_t("p2de", K2D), pos_t("p2do", K2D)

    out = nc.dram_tensor("out", [RPAD, D_OUT], dt.float32, kind="ExternalOutput")

    z1A = nc.dram_tensor("z1A", [SPLIT1, D_HID], dt.float8e4)
    z1B = nc.dram_tensor("z1B", [NPAD - SPLIT1, D_HID], dt.float8e4)
    # z2 rows unpadded (64 cols bf16 = 128B); the gather fetches PAIRS of
    # rows per 256B descriptor via a [rows/2, 128] view of these tensors.
    z2in0 = nc.dram_tensor("z2in0", [S0_ROWS, D_OUT], dt.bfloat16)
    z2in1 = nc.dram_tensor("z2in1", [S1_ROWS, D_OUT], dt.bfloat16)
    z2out0 = nc.dram_tensor("z2out0", [NCORES * S0_ROWS, D_OUT], dt.bfloat16,
                            addr_space="Shared")
    z2out1 = nc.dram_tensor("z2out1", [NCORES * S1_ROWS, D_OUT], dt.bfloat16,
                            addr_space="Shared")

    qctr = [0]

    def next_q():
        q = qctr[0] % NQ
        qctr[0] += 1
        return q

    with tile.TileContext(nc) as tc:
        with tc.tile_pool(name="consts", bufs=1) as cp, \
             tc.tile_pool(name="ph0x", bufs=2) as xp, \
             tc.tile_pool(name="ph0o", bufs=3) as op0, \
             tc.tile_pool(name="gp", bufs=8) as gp, \
             tc.tile_pool(name="gp2", bufs=8) as gp2, \
             tc.tile_pool(name="ohp", bufs=3) as ohp, \
             tc.tile_pool(name="csp", bufs=BPC + 1) as csp, \
             tc.tile_pool(name="hp", bufs=2) as hp, \
             tc.tile_pool(name="zp", bufs=3) as zp, \
             tc.tile_pool(name="smp", bufs=8) as smp, \
             tc.tile_pool(name="psAcc", bufs=3, space="PSUM") as psAcc, \
             tc.tile_pool(name="psMisc", bufs=1, space="PSUM") as psMisc, \
             tc.tile_pool(name="psO", bufs=3, space="PSUM") as psO:
            w1t = cp.tile([128, 4, D_HID], dt.float8e4)
            nc.sync.dma_start(
                w1t[:], w1.ap().rearrange("(k p) n -> p k n", p=128))
            w2t = cp.tile([128, 2, D_OUT], dt.bfloat16)
            nc.sync.dma_start(
                w2t[:], w2.ap().rearrange("(k p) n -> p k n", p=128))
            b1r = cp.tile([128, D_HID], dt.float32)
            nc.sync.dma_start(b1r[:], b1rep[:, :])
            idt = cp.tile([128, 128], dt.bfloat16)
            nc.sync.dma_start(idt[:], ident[:, :])
            b2t = cp.tile([128, D_OUT], dt.float32)
            nc.sync.dma_start(b2t[:], b2rep[:, :])
            dvt = cp.tile([128, BPC], dt.float32)
            nc.sync.dma_start(dvt[:], dinvb[:, :])
            it1a = cp.tile([128, BPC * K1A * BLK // 16], dt.int16)
            nc.scalar.dma_start(it1a[:], i1a[:, :])
            it1b = cp.tile([128, BPC * K1B * BLK // 16], dt.int16)
            nc.scalar.dma_start(it1b[:], i1b[:, :])
            it2c = cp.tile([128, BPC * K2C * BLK // 16], dt.int16)
            nc.scalar.dma_start(it2c[:], i2c[:, :])
            it2d = cp.tile([128, BPC * K2D * BLK // 16], dt.int16)
            nc.scalar.dma_start(it2d[:], i2d[:, :])
            pt1a = cp.tile([128, BPC * K1A], dt.int16)
            nc.scalar.dma_start(pt1a[:], p1a[:, :])
            pt1b = cp.tile([128, BPC * K1B], dt.int16)
            nc.scalar.dma_start(pt1b[:], p1b[:, :])
            pt2ce = cp.tile([128, BPC * K2C], dt.int16)
            nc.scalar.dma_start(pt2ce[:], p2ce[:, :])
            pt2co = cp.tile([128, BPC * K2C], dt.int16)
            nc.scalar.dma_start(pt2co[:], p2co[:, :])
            pt2de = cp.tile([128, BPC * K2D], dt.int16)
            nc.scalar.dma_start(pt2de[:], p2de[:, :])
            pt2do = cp.tile([128, BPC * K2D], dt.int16)
            nc.scalar.dma_start(pt2do[:], p2do[:, :])
            iot = cp.tile([128, 128], dt.int16)
            nc.gpsimd.iota(iot[:], [[1, 128]], base=0, channel_multiplier=0)
            iot3 = iot[:].rearrange("p (o j) -> p o j", o=1)

            # ------- phase 0: z1 = xT^T @ W1, fp8 DoubleRow -------
            z1Av = z1A.ap().rearrange("(n p) f -> p n f", p=128)
            z1Bv = z1B.ap().rearrange("(n p) f -> p n f", p=128)
            NB_A = SPLIT1 // BLK
            GB = 7
            for g0 in range(0, NBLOCKS, GRP):
                gb = min(GRP, NBLOCKS - g0)
                xg = xp.tile([128, 4, GRP * BLK], dt.float8e4, tag="xg")
                nc.sync.dma_start(
                    xg[:, :, :gb * BLK],
                    xTg.ap()[g0 // GRP, :, :, :gb * BLK])
                for b0 in range(0, gb, GB):
                    nb = min(GB, gb - b0)
                    zo = op0.tile([128, GB, D_HID], dt.float8e4, tag="zo")
                    for i in range(nb):
                        ps = psAcc.tile([128, D_HID], dt.float32, tag="acc")
                        col = (b0 + i) * BLK
                        for t in range(2):
                            nc.tensor.matmul(
                                ps[:],
                                xg[:, 2 * t:2 * t + 2, col:col + BLK],
                                w1t[:, 2 * t:2 * t + 2, :],
                                start=(t == 0), stop=(t == 1),
                                perf_mode=DR)
                        nc.vector.tensor_copy(zo[:, i, :], ps[:])
                    lo, hi = g0 + b0, g0 + b0 + nb
                    if hi <= NB_A:
                        nc.sync.dma_start(z1Av[:, lo:hi, :], zo[:, :nb, :])
                    elif lo >= NB_A:
                        nc.sync.dma_start(
                            z1Bv[:, lo - NB_A:hi - NB_A, :], zo[:, :nb, :])
                    else:
                        na = NB_A - lo
                        nc.sync.dma_start(z1Av[:, lo:NB_A, :], zo[:, :na, :])
                        nc.sync.dma_start(
                            z1Bv[:, 0:hi - NB_A, :], zo[:, na:nb, :])

            # ---------------- phases 1+2 ----------------
            seg1 = {
                "A": (K1A, it1a, pt1a, z1A.ap()[:, :]),
                "B": (K1B, it1b, pt1b, z1B.ap()[:, :]),
            }
            seg2 = {
                "C": (K2C, it2c, (pt2ce, pt2co),
                      z2out0.ap().rearrange("(r two) f -> r (two f)", two=2)),
                "D": (K2D, it2d, (pt2de, pt2do),
                      z2out1.ap().rearrange("(r two) f -> r (two f)", two=2)),
            }
            gtiles = {}

            def ensure_g(layer, s, pi):
                key = (layer, s, pi)
                if key in gtiles:
                    return gtiles[key]
                K, itile, _, zview = (seg1 if layer == 1 else seg2)[s]
                felem = D_HID if layer == 1 else 2 * D_OUT
                fdt = dt.float8e4 if layer == 1 else dt.bfloat16
                SL = BPC * K * BLK
                n = min(PIECE, SL - pi * PIECE)
                off = pi * (PIECE // 16)
                pool = gp if layer == 1 else gp2
                gt = pool.tile([128, CHPP, felem], fdt, tag=f"g{layer}{s}")
                nc.gpsimd.dma_gather(
                    gt[:, :n // 128, :], zview, itile[:, off:off + n // 16],
                    n, n, felem, queue_num=next_q())
                gtiles[key] = gt
                return gt

            def make_onehot_t(ptile, K, b, odt, tag):
                oh = ohp.tile([128, K, 128], odt, tag=tag)
                pv = ptile[:, b * K:(b + 1) * K].rearrange(
                    "p (c o) -> p c o", o=1)
                a0, a1 = broadcast_tensor_aps(iot3, pv)
                nc.vector.tensor_tensor(
                    oh[:], a0, a1, op=mybir.AluOpType.is_equal)
                return oh

            def make_onehot(layer, s, b):
                K, _, ptile, _ = seg1[s]
                return make_onehot_t(ptile, K, b, dt.float8e4, f"oh{layer}{s}")

            def l1_block(b):
                hps = psAcc.tile([128, D_HID], dt.float32, tag="acc")
                first = True
                for s in ("A", "B"):
                    K = seg1[s][0]
                    oh = make_onehot(1, s, b)
                    k = 0
                    while k < K:
                        ci = b * K + k
                        pi, cpos = divmod(ci, CHPP)
                        last_s = (s == "B")
                        gt = ensure_g(1, s, pi)
                        if k + 1 < K and cpos + 1 < CHPP:
                            nc.tensor.matmul(
                                hps[:], oh[:, k:k + 2, :],
                                gt[:, cpos:cpos + 2, :],
                                start=first, stop=(last_s and k + 2 == K),
                                perf_mode=DR)
                            k += 2
                        else:
                            nc.tensor.matmul(
                                hps[:], oh[:, k, :], gt[:, cpos, :],
                                start=first, stop=(last_s and k + 1 == K))
                            k += 1
                        first = False
                hs = hp.tile([128, D_HID], dt.float32, tag="hs")
                nc.vector.tensor_scalar(
                    hs[:], hps[:], dvt[:, b:b + 1], None,
                    op0=mybir.AluOpType.mult)
                hb = hp.tile([128, D_HID], dt.bfloat16, tag="hb")
                nc.vector.tensor_tensor(
                    hb[:], hs[:], b1r[:], op=mybir.AluOpType.add)
                hr = hp.tile([128, D_HID], dt.bfloat16, tag="hr")
                nc.scalar.activation(
                    hr[:], hb[:], mybir.ActivationFunctionType.Relu)
                hT = hp.tile([128, 2, 128], dt.bfloat16, tag="hT")
                for h in range(2):
                    tps = psMisc.tile([128, 128], dt.bfloat16, tag="tps")
                    nc.tensor.transpose(
                        tps[:], hr[:, h * 128:(h + 1) * 128], idt[:])
                    nc.scalar.copy(hT[:, h, :], tps[:])
                zps = psMisc.tile([128, D_OUT], dt.float32, tag="zps")
                for h in range(2):
                    nc.tensor.matmul(
                        zps[:], hT[:, h, :], w2t[:, h, :],
                        start=(h == 0), stop=(h == 1))
                z2s = zp.tile([128, D_OUT], dt.bfloat16, tag="z2s")
                nc.vector.tensor_scalar(
                    z2s[:], zps[:], dvt[:, b:b + 1], None,
                    op0=mybir.AluOpType.mult)
                if b < S0_BLOCKS:
                    nc.sync.dma_start(
                        z2in0.ap()[b * BLK:(b + 1) * BLK, :], z2s[:])
                else:
                    bb = b - S0_BLOCKS
                    nc.sync.dma_start(
                        z2in1.ap()[bb * BLK:(bb + 1) * BLK, :], z2s[:])

            cstash = {}

            def l2cd_block(b, s):
                ops = psO.tile([128, D_OUT], dt.float32, tag="ops")
                K, _, (pte, pto), _ = seg2[s]
                ohe = make_onehot_t(pte, K, b, dt.bfloat16, f"oh2{s}e")
                oho = make_onehot_t(pto, K, b, dt.bfloat16, f"oh2{s}o")
                for k in range(K):
                    ci = b * K + k
                    pi, cpos = divmod(ci, CHPP)
                    gt = ensure_g(2, s, pi)
                    nc.tensor.matmul(
                        ops[:], ohe[:, k, :], gt[:, cpos, :D_OUT],
                        start=(k == 0), stop=False)
                    nc.tensor.matmul(
                        ops[:], oho[:, k, :], gt[:, cpos, D_OUT:],
                        start=False, stop=(k == K - 1))
                return ops

            def l2c_block(b):
                ops = l2cd_block(b, "C")
                cs0 = zp.tile([128, D_OUT], dt.float32, tag="cs0")
                nc.scalar.activation(
                    cs0[:], ops[:], mybir.ActivationFunctionType.Copy,
                    scale=dvt[:, b:b + 1])
                cs = csp.tile([128, D_OUT], dt.float32, tag="cs")
                nc.vector.tensor_tensor(
                    cs[:], cs0[:], b2t[:], op=mybir.AluOpType.add)
                cstash[b] = cs

            def l2d_block(b):
                ops = l2cd_block(b, "D")
                t = smp.tile([128, D_OUT], dt.float32, tag="t")
                nc.vector.tensor_scalar(
                    t[:], ops[:], dvt[:, b:b + 1], None,
                    op0=mybir.AluOpType.mult)
                t2 = smp.tile([128, D_OUT], dt.float32, tag="t2")
                nc.vector.tensor_tensor(
                    t2[:], t[:], cstash[b][:], op=mybir.AluOpType.add)
                nm = smp.tile([128, 1], dt.float32, tag="nm")
                nc.vector.reduce_max(
                    nm[:], t2[:], axis=mybir.AxisListType.X, negate=True)
                ex = smp.tile([128, D_OUT], dt.float32, tag="ex")
                sm = smp.tile([128, 1], dt.float32, tag="sm")
                nc.scalar.activation(
                    ex[:], t2[:], mybir.ActivationFunctionType.Exp,
                    bias=nm[:], accum_out=sm[:])
                rc = smp.tile([128, 1], dt.float32, tag="rc")
                nc.vector.reciprocal(rc[:], sm[:])
                ot = smp.tile([128, D_OUT], dt.float32, tag="ot")
                nc.vector.tensor_scalar(
                    ot[:], ex[:], rc[:], None, op0=mybir.AluOpType.mult)
                nc.sync.dma_start(out.ap()[b * BLK:(b + 1) * BLK, :], ot[:])

            if phases >= 1:
                for b in range(32):
                    l1_block(b)
                if phases >= 2:
                    nc.gpsimd.collective_compute(
                        "AllGather", mybir.AluOpType.bypass,
                        replica_groups=[list(range(NCORES))],
                        ins=[z2in0.ap().opt()], outs=[z2out0.ap().opt()])
                ci = 0
                for b in range(32, BPC):
                    l1_block(b)
                    if phases >= 3 and b >= 40 and ci < BPC:
                        l2c_block(ci)
                        ci += 1
                if phases >= 2:
                    nc.gpsimd.collective_compute(
                        "AllGather", mybir.AluOpType.bypass,
                        replica_groups=[list(range(NCORES))],
                        ins=[z2in1.ap().opt()], outs=[z2out1.ap().opt()])
                if phases >= 3:
                    while ci < BPC:
                        l2c_block(ci)
                        ci += 1
                    for b in range(BPC):
                        l2d_block(b)

    nc.compile()

    # The Tile scheduler reorders instructions; DMASW sem lanes rotate mod 8
    # over Pool-engine DMAs in FINAL order and each sem may only be driven by
    # one SWDGE queue.  Rewrite queue_num to lane%NQ in final order so the
    # lane<->queue pairing is always consistent.
    if os.environ.get("GCN_QFIX", "1") == "1":
        from concourse.tile_scheduler import DMAInst
        idx = 0
        for blk in nc.m.functions[0].blocks:
            for inst in blk.instructions:
                if (inst.engine == mybir.EngineType.Pool
                        and isinstance(inst, DMAInst)):
                    inst.queue_num = (idx % 8) % NQ
                    idx += 1
    return nc


# ---------------- entry point ----------------

def kernel(x, edge_index, W1, b1, W2, b2):
    x = np.asarray(x)
    edge_index = np.asarray(edge_index)
    in_maps, perms, Ks = _preprocess(x, edge_index, W1, b1, W2, b2)
    nc = _build_program(*Ks)

    trace = os.environ.get("GCN_TRACE", "0") == "1"
    if trace:
        trace = _install_trace_hook()
    res = run_bass_kernel_spmd(
        nc, in_maps, core_ids=list(range(NCORES)), trace=trace)
    LAST["exec_time_ns"] = res.exec_time_ns
    LAST["results"] = res

    out = np.empty((N_NODES, D_OUT), dtype=np.float32)
    for c in range(NCORES):
        oc = np.asarray(res.results[c]["out"], dtype=np.float32)
        out[c * RPC:(c + 1) * RPC] = oc[perms[c]]
    return out
